# revision 5
# baseline (speedup 1.0000x reference)
"""3-layer GCN + linear head on 8 Trainium2 NeuronCores.

Sharding: nodes are partitioned across the 8 cores (graph parallel), after a
host-side balanced permutation that gives every 128-node block exactly the
same number of incoming edges (including self loops).  Per layer each core:
  - transforms its 1024 rows (dense matmul, weights replicated),
  - AllGathers the transformed rows to every core,
  - gathers edge-source rows with SWDGE dma_gather and reduces them into
    destination rows with TensorE matmuls against host-built per-chunk
    selection matrices S (which carry the GCN edge normalization weights).
Layer 1 aggregates x first (256-dim) and transforms after, which is cheaper.
All arithmetic is fp32; accumulation in PSUM.
"""
import sys

if "/opt/trn_rl_repo" not in sys.path:
    sys.path.insert(0, "/opt/trn_rl_repo")

import numpy as np

import concourse.bass as bass
import concourse.mybir as mybir
import concourse.tile as tile
from concourse import bacc
from concourse.bass_utils import run_bass_kernel_spmd
from concourse.library_config import mlp

N = 8192
NUM_CORES = 8
R = N // NUM_CORES          # rows per core
NB = 8                      # dst blocks per core (128 rows each)
NBINS = NUM_CORES * NB
BIN_SZ = 128
F_IN, H1, H2, H3, F_OUT = 256, 2048, 2048, 1024, 768
DT = mybir.dt.float32
TANH = mybir.ActivationFunctionType.Tanh


# ----------------------------------------------------------------------------
# Host-side graph preprocessing
# ----------------------------------------------------------------------------

def _preprocess(edge_index):
    src = np.asarray(edge_index[0], dtype=np.int64)
    dst = np.asarray(edge_index[1], dtype=np.int64)

    deg = np.bincount(dst, minlength=N).astype(np.float64) + 1.0
    dinv = 1.0 / np.sqrt(deg)
    d_in = np.bincount(dst, minlength=N) + 1

    # greedy balanced partition of nodes into bins of 128, equal in-edge sums
    order = np.argsort(-d_in, kind="stable")
    bin_sum = np.zeros(NBINS, dtype=np.int64)
    bin_cnt = np.zeros(NBINS, dtype=np.int64)
    bin_nodes = [[] for _ in range(NBINS)]
    for node in order:
        avail = np.where(bin_cnt < BIN_SZ)[0]
        b = avail[np.argmin(bin_sum[avail])]
        bin_nodes[b].append(node)
        bin_sum[b] += d_in[node]
        bin_cnt[b] += 1

    target = int(np.ceil(d_in.sum() / NBINS))
    for _ in range(200):
        hi = int(np.argmax(bin_sum))
        if bin_sum[hi] <= target:
            break
        lo = int(np.argmin(bin_sum))
        need = bin_sum[hi] - target
        best = None
        for ai, a in enumerate(bin_nodes[hi]):
            for bi, b in enumerate(bin_nodes[lo]):
                diff = d_in[a] - d_in[b]
                if diff > 0:
                    score = abs(diff - need)
                    if best is None or score < best[0]:
                        best = (score, ai, bi)
        if best is None:
            break
        _, ai, bi = best
        a, b = bin_nodes[hi][ai], bin_nodes[lo][bi]
        bin_nodes[hi][ai], bin_nodes[lo][bi] = b, a
        bin_sum[hi] += d_in[b] - d_in[a]
        bin_sum[lo] += d_in[a] - d_in[b]

    CH = int(np.ceil(bin_sum.max() / 128))
    EPB = CH * 128

    perm = np.concatenate([np.array(bn, dtype=np.int64) for bn in bin_nodes])
    inv = np.empty(N, dtype=np.int64)
    inv[perm] = np.arange(N)

    all_src = np.concatenate([inv[src], np.arange(N, dtype=np.int64)])
    all_dst = np.concatenate([inv[dst], np.arange(N, dtype=np.int64)])
    all_w = np.concatenate([
        (dinv[src] * dinv[dst]).astype(np.float32),
        (dinv[perm] * dinv[perm]).astype(np.float32),
    ])

    bin_of = all_dst // BIN_SZ
    dst_local = all_dst % BIN_SZ

    TOT_CH = NB * CH
    idx_tabs, s_mats = [], []
    arange_epb = np.arange(EPB)
    for c in range(NUM_CORES):
        idx_tab = np.zeros((128, TOT_CH * 8), np.int16)
        s_mat = np.zeros((128, TOT_CH, 128), np.float32)
        for blk in range(NB):
            sel = np.where(bin_of == c * NB + blk)[0]
            n_e = len(sel)
            e_src = np.zeros(EPB, np.int64)
            e_dst = np.zeros(EPB, np.int64)
            e_w = np.zeros(EPB, np.float32)
            e_src[:n_e] = all_src[sel]
            e_dst[:n_e] = dst_local[sel]
            e_w[:n_e] = all_w[sel]
            idx_tab[arange_epb % 16, blk * CH * 8 + arange_epb // 16] = \
                e_src.astype(np.int16)
            s_mat[arange_epb % 128, blk * CH + arange_epb // 128, e_dst] = e_w
        for rep in range(1, 8):
            idx_tab[rep * 16:(rep + 1) * 16, :] = idx_tab[:16, :]
        idx_tabs.append(idx_tab)
        s_mats.append(s_mat)

    return perm, CH, idx_tabs, s_mats


# ----------------------------------------------------------------------------
# Device program
# ----------------------------------------------------------------------------

def _build_program(CH, repeat=1, phase_reps=None, mm_dt=DT, no_cc=False):
    """Build the SPMD program.  `repeat` repeats the whole pipeline (for
    timing); `phase_reps` maps phase name -> extra repetitions (timing only:
    a repeated phase recomputes the same values)."""
    pr = dict(L1=1, L2T=1, AG2=1, L2A=1, L3T=1, AG3=1, L3A=1, FIN=1)
    if phase_reps:
        pr.update(phase_reps)
    TOT_CH = NB * CH
    nc = bacc.Bacc("TRN2", target_bir_lowering=False, debug=False,
                   num_devices=NUM_CORES)
    core_ids = list(range(NUM_CORES))

    x_perm = nc.dram_tensor("x_perm", [N, F_IN], mm_dt, kind="ExternalInput")
    idx_in = nc.dram_tensor("idx_in", [128, TOT_CH * 8], mybir.dt.int16,
                            kind="ExternalInput")
    s_in = nc.dram_tensor("s_in", [128, TOT_CH, 128], mm_dt, kind="ExternalInput")
    ident_in = nc.dram_tensor("ident", [128, 128], mm_dt, kind="ExternalInput")
    # weights pre-tiled on host to [128, K/128, F] layout
    w1t_in = nc.dram_tensor("w1t", [128, F_IN // 128, H1], mm_dt, kind="ExternalInput")
    w2t_in = nc.dram_tensor("w2t", [128, H1 // 128, H2], mm_dt, kind="ExternalInput")
    w3t_in = nc.dram_tensor("w3t", [128, H2 // 128, H3], mm_dt, kind="ExternalInput")
    wlt_in = nc.dram_tensor("wlt", [128, H3 // 128, F_OUT], mm_dt, kind="ExternalInput")
    b1_in = nc.dram_tensor("b1pp", [128, H1 // 128], DT, kind="ExternalInput")
    b2_in = nc.dram_tensor("b2pp", [128, H2 // 128], DT, kind="ExternalInput")
    b3_in = nc.dram_tensor("b3pp", [128, H3 // 128], DT, kind="ExternalInput")
    bl_in = nc.dram_tensor("blb", [128, F_OUT], DT, kind="ExternalInput")

    out = nc.dram_tensor("out", [R, F_OUT], DT, kind="ExternalOutput")

    xw2_locs = [nc.dram_tensor(f"xw2_loc{h}", [R, H2 // 2], mm_dt)
                for h in range(2)]
    xw2_fulls = [nc.dram_tensor(f"xw2_full{h}", [N, H2 // 2], mm_dt,
                                addr_space="Shared") for h in range(2)]
    xw3_locs = [nc.dram_tensor(f"xw3_loc{h}", [R, H3 // 2], mm_dt)
                for h in range(2)]
    xw3_fulls = [nc.dram_tensor(f"xw3_full{h}", [N, H3 // 2], mm_dt,
                                addr_space="Shared") for h in range(2)]

    uid = [0]

    def pname(base):
        uid[0] += 1
        return f"{base}{uid[0]}"

    with tile.TileContext(nc) as tc:
        with tc.tile_pool(name="const", bufs=1) as cpool:
            nc.gpsimd.load_library(mlp)
            idx_sb = cpool.tile([128, TOT_CH * 8], mybir.dt.int16, tag="idx")
            nc.sync.dma_start(idx_sb[:], idx_in[:])
            s_sb = cpool.tile([128, TOT_CH, 128], mm_dt, tag="s")
            nc.sync.dma_start(s_sb[:], s_in[:])
            id_sb = cpool.tile([128, 128], mm_dt, tag="ident")
            nc.sync.dma_start(id_sb[:], ident_in[:])
            b1_sb = cpool.tile([128, H1 // 128], DT, tag="b1")
            nc.sync.dma_start(b1_sb[:], b1_in[:])
            b2_sb = cpool.tile([128, H2 // 128], DT, tag="b2")
            nc.sync.dma_start(b2_sb[:], b2_in[:])
            b3_sb = cpool.tile([128, H3 // 128], DT, tag="b3")
            nc.sync.dma_start(b3_sb[:], b3_in[:])
            bl_sb = cpool.tile([128, F_OUT], DT, tag="bl")
            nc.sync.dma_start(bl_sb[:], bl_in[:])

            def phase_L1(h1t):
                """gather x, aggregate node-major, transpose, transform+tanh."""
                with (
                    tc.tile_pool(name=pname("l1a"), bufs=1) as l1a_pool,
                    tc.tile_pool(name=pname("l1ps"), bufs=1, space="PSUM") as l1ps,
                ):
                    w1t_sb = l1a_pool.tile([128, F_IN // 128, H1], mm_dt, tag="w1t")
                    nc.sync.dma_start(w1t_sb[:], w1t_in[:])
                    agg1t = l1a_pool.tile([128, F_IN // 128, R], mm_dt, tag="agg1t")
                    for blk in range(NB):
                        g = l1a_pool.tile([128, CH, F_IN], mm_dt, tag="g1", bufs=2)
                        nc.gpsimd.dma_gather(
                            g[:], x_perm[:],
                            idx_sb[:, blk * CH * 8:(blk + 1) * CH * 8],
                            CH * 128, CH * 128, F_IN, single_packet=False)
                        ps = l1ps.tile([128, F_IN], DT, tag="agg", bufs=2)
                        for c in range(CH):
                            nc.tensor.matmul(
                                ps[:, :], s_sb[:, blk * CH + c, :], g[:, c, :],
                                start=(c == 0), stop=(c == CH - 1))
                        a_nm = l1a_pool.tile([128, F_IN], mm_dt, tag="anm", bufs=2)
                        nc.vector.tensor_copy(a_nm[:], ps[:])
                        for f in range(F_IN // 128):
                            pt = l1ps.tile([128, 128], mm_dt, tag="pt", bufs=2)
                            nc.tensor.transpose(
                                pt[:], a_nm[:, f * 128:(f + 1) * 128], id_sb[:])
                            nc.vector.tensor_copy(
                                agg1t[:, f, blk * 128:(blk + 1) * 128], pt[:])
                    for m in range(H1 // 128):
                        ps = l1ps.tile([128, R], DT, tag="xw", bufs=2)
                        for k in range(F_IN // 128):
                            for n in range(0, R, 512):
                                nc.tensor.matmul(
                                    ps[:, n:n + 512],
                                    w1t_sb[:, k, m * 128:(m + 1) * 128],
                                    agg1t[:, k, n:n + 512],
                                    start=(k == 0), stop=(k == F_IN // 128 - 1))
                        nc.scalar.activation(
                            h1t[:, m, :], ps[:], TANH, bias=b1_sb[:, m:m + 1])

            def transform(ht, KD, FD, wt_in, locs, fulls, halves):
                """locs[h][:, :] = (ht rows) @ W^T half h, node-major; AllGather
                each half as soon as it is written so the collective overlaps
                with the next half's matmuls."""
                HK = KD // 128
                with (
                    tc.tile_pool(name=pname("tr"), bufs=1) as tpool,
                    tc.tile_pool(name=pname("trps"), bufs=1, space="PSUM") as tps,
                ):
                    FH = FD // halves
                    dsz = mybir.dt.size(mm_dt)
                    for h in range(halves):
                        wt_bufs = 2 if HK * FH * dsz <= 32 * 1024 else 1
                        wt_sb = tpool.tile([128, HK, FH], mm_dt, tag="wt",
                                           bufs=wt_bufs)
                        nc.sync.dma_start(
                            wt_sb[:], wt_in[:, :, h * FH:(h + 1) * FH])
                        for r in range(NB):
                            ps = tps.tile([128, FH], DT, tag="xw", bufs=2)
                            for k in range(HK):
                                for n0 in range(0, FH, 512):
                                    n1 = min(n0 + 512, FH)
                                    nc.tensor.matmul(
                                        ps[:, n0:n1],
                                        ht[:, k, r * 128:(r + 1) * 128],
                                        wt_sb[:, k, n0:n1],
                                        start=(k == 0), stop=(k == HK - 1))
                            o = tpool.tile([128, FH], mm_dt, tag="o", bufs=3)
                            nc.vector.tensor_copy(o[:], ps[:])
                            nc.sync.dma_start(
                                locs[h][r * 128:(r + 1) * 128, :], o[:])
                        if no_cc:
                            for cb in range(NUM_CORES):
                                nc.sync.dma_start(
                                    fulls[h][cb * R:(cb + 1) * R, :],
                                    locs[h][:])
                        else:
                            nc.gpsimd.collective_compute(
                                "AllGather", mybir.AluOpType.bypass,
                                replica_groups=[core_ids],
                                ins=[locs[h][:]], outs=[fulls[h][:]])

            def aggregate(fulls, FD, ht, b_sb):
                """gather rows of each AllGathered half by edge sources,
                node-major reduce, transpose + tanh(.+bias) into feature-major
                ht.  Halves gate independently on their own collective."""
                halves = len(fulls)
                FH = FD // halves
                with (
                    tc.tile_pool(name=pname("ag"), bufs=1) as apool,
                    tc.tile_pool(name=pname("agps"), bufs=1, space="PSUM") as aps,
                ):
                    agg_bufs = 1 if FD * 4 // 512 >= 8 else 2
                    GB = min(CH, max(1, 12288 // (FH * 4 // 128) // 128))
                    for blk in range(NB):
                        ps = aps.tile([128, FD], DT, tag="agg", bufs=agg_bufs)
                        for c0 in range(0, CH, GB):
                            gb = min(GB, CH - c0)
                            t0 = blk * CH + c0
                            for h in range(halves):
                                g = apool.tile([128, GB, FH], mm_dt,
                                               tag=f"g{h}", bufs=2)
                                nc.gpsimd.dma_gather(
                                    g[:, :gb, :], fulls[h][:],
                                    idx_sb[:, t0 * 8:(t0 + gb) * 8],
                                    gb * 128, gb * 128, FH,
                                    single_packet=False)
                                for j in range(gb):
                                    c = c0 + j
                                    for nf in range(FH // 512):
                                        nc.tensor.matmul(
                                            ps[:, h * FH + nf * 512:
                                               h * FH + (nf + 1) * 512],
                                            s_sb[:, blk * CH + c, :],
                                            g[:, j, nf * 512:(nf + 1) * 512],
                                            start=(c == 0), stop=(c == CH - 1))
                        a_nm = apool.tile([128, FD], mm_dt, tag="anm", bufs=2)
                        nc.vector.tensor_copy(a_nm[:], ps[:])
                        for f in range(FD // 128):
                            pt = aps.tile([128, 128], mm_dt, tag="pt", bufs=3)
                            nc.tensor.transpose(
                                pt[:], a_nm[:, f * 128:(f + 1) * 128], id_sb[:])
                            nc.scalar.activation(
                                ht[:, f, blk * 128:(blk + 1) * 128], pt[:],
                                TANH, bias=b_sb[:, f:f + 1])

            def phase_FIN(h3t):
                with (
                    tc.tile_pool(name=pname("fin"), bufs=1) as fpool,
                    tc.tile_pool(name=pname("finps"), bufs=1, space="PSUM") as fps,
                ):
                    wlt_sb = fpool.tile([128, H3 // 128, F_OUT], mm_dt, tag="wlt")
                    nc.sync.dma_start(wlt_sb[:], wlt_in[:])
                    for r in range(NB):
                        ps = fps.tile([128, F_OUT], DT, tag="xw", bufs=2)
                        for k in range(H3 // 128):
                            for n0 in range(0, F_OUT, 512):
                                n1 = min(n0 + 512, F_OUT)
                                nc.tensor.matmul(
                                    ps[:, n0:n1],
                                    h3t[:, k, r * 128:(r + 1) * 128],
                                    wlt_sb[:, k, n0:n1],
                                    start=(k == 0), stop=(k == H3 // 128 - 1))
                        o = fpool.tile([128, F_OUT], DT, tag="o", bufs=3)
                        nc.vector.tensor_tensor(
                            out=o[:], in0=ps[:], in1=bl_sb[:],
                            op=mybir.AluOpType.add)
                        nc.sync.dma_start(out[r * 128:(r + 1) * 128, :], o[:])

            for rep in range(repeat):
                with tc.tile_pool(name=pname("h1t"), bufs=1) as h1t_pool:
                    h1t = h1t_pool.tile([128, H1 // 128, R], mm_dt, tag="h1t")
                    for _ in range(pr["L1"]):
                        phase_L1(h1t)
                    for _ in range(pr["L2T"]):
                        transform(h1t, H1, H2, w2t_in, xw2_locs, xw2_fulls,
                                  halves=2)
                with tc.tile_pool(name=pname("h2t"), bufs=1) as h2t_pool:
                    h2t = h2t_pool.tile([128, H2 // 128, R], mm_dt, tag="h2t")
                    for _ in range(pr["L2A"]):
                        aggregate(xw2_fulls, H2, h2t, b2_sb)
                    for _ in range(pr["L3T"]):
                        transform(h2t, H2, H3, w3t_in, xw3_locs, xw3_fulls,
                                  halves=2)
                with tc.tile_pool(name=pname("h3t"), bufs=1) as h3t_pool:
                    h3t = h3t_pool.tile([128, H3 // 128, R], mm_dt, tag="h3t")
                    for _ in range(pr["L3A"]):
                        aggregate(xw3_fulls, H3, h3t, b3_sb)
                    for _ in range(pr["FIN"]):
                        phase_FIN(h3t)

    nc.compile()
    return nc


# ----------------------------------------------------------------------------
# Entry point
# ----------------------------------------------------------------------------

def _make_in_maps(inputs, perm, idx_tabs, s_mats, np_mm_dt=None):
    if np_mm_dt is None:
        import ml_dtypes
        np_mm_dt = ml_dtypes.bfloat16

    def tile_w(w):  # [K, F] -> [128, K/128, F]
        k, f = w.shape
        return np.ascontiguousarray(
            w.reshape(k // 128, 128, f).transpose(1, 0, 2)).astype(np_mm_dt)

    x_perm = np.ascontiguousarray(
        np.asarray(inputs["x"], np.float32)[perm]).astype(np_mm_dt)
    w1t = tile_w(np.ascontiguousarray(np.asarray(inputs["W1"], np.float32).T))
    w2t = tile_w(np.ascontiguousarray(np.asarray(inputs["W2"], np.float32).T))
    w3t = tile_w(np.ascontiguousarray(np.asarray(inputs["W3"], np.float32).T))
    wlt = tile_w(np.ascontiguousarray(np.asarray(inputs["Wl"], np.float32).T))
    b1pp = np.ascontiguousarray(
        np.asarray(inputs["b1"], np.float32).reshape(-1, 128).T)
    b2pp = np.ascontiguousarray(
        np.asarray(inputs["b2"], np.float32).reshape(-1, 128).T)
    b3pp = np.ascontiguousarray(
        np.asarray(inputs["b3"], np.float32).reshape(-1, 128).T)
    blb = np.ascontiguousarray(
        np.broadcast_to(np.asarray(inputs["bl"], np.float32), (128, F_OUT)))
    ident = np.eye(128, dtype=np_mm_dt)

    in_maps = []
    for c in range(NUM_CORES):
        in_maps.append({
            "x_perm": x_perm, "idx_in": idx_tabs[c],
            "s_in": s_mats[c].astype(np_mm_dt),
            "ident": ident,
            "w1t": w1t, "w2t": w2t, "w3t": w3t, "wlt": wlt,
            "b1pp": b1pp, "b2pp": b2pp, "b3pp": b3pp, "blb": blb,
        })
    return in_maps


def _run(inputs, trace=False):
    perm, CH, idx_tabs, s_mats = _preprocess(np.asarray(inputs["edge_index"]))
    nc = _build_program(CH, mm_dt=mybir.dt.bfloat16)
    in_maps = _make_in_maps(inputs, perm, idx_tabs, s_mats)
    res = run_bass_kernel_spmd(nc, in_maps, list(range(NUM_CORES)), trace=trace)
    out_perm = np.concatenate([res.results[c]["out"] for c in range(NUM_CORES)], 0)
    out = np.empty_like(out_perm)
    out[perm] = out_perm
    return out, res


def kernel(**inputs):
    out, _ = _run(inputs, trace=False)
    return out



# revision 10
# speedup vs baseline: 1.0610x; 1.0610x over previous
"""3-layer GCN + linear head on 8 Trainium2 NeuronCores.

Sharding: nodes are partitioned across the 8 cores (graph parallel), after a
host-side balanced permutation that gives every 128-node block exactly the
same number of incoming edges (including self loops).  All message traffic is
bf16 (tolerance is 2e-2); PSUM accumulation is fp32.

Per layer (2, 3) each core:
  - transforms its local rows 0-511 (dense matmul, weights replicated),
    AllGathers them into fullA [4096, FD] while transforming rows 512-1023,
    which AllGather into fullB,
  - gathers edge-source rows (full-width, 1 DMA descriptor per edge) with
    SWDGE dma_gather and reduces them into destination rows with TensorE
    matmuls against host-built per-chunk selection matrices S (which carry
    the GCN edge normalization weights).  All A-half gathers for the 8 dst
    blocks run first (they only gate on AllGather #1, overlapping AllGather
    #2); partial sums park in SBUF and are folded in during the B half.
Layer 1 aggregates x first (256-dim messages) and transforms after.  The
final linear head has no aggregation.
"""
import sys

if "/opt/trn_rl_repo" not in sys.path:
    sys.path.insert(0, "/opt/trn_rl_repo")

import numpy as np

import concourse.bass as bass
import concourse.mybir as mybir
import concourse.tile as tile
from concourse import bacc
from concourse.bass_utils import run_bass_kernel_spmd
from concourse.library_config import mlp

N = 8192
NUM_CORES = 8
R = N // NUM_CORES          # rows per core
HALF = R // 2               # rows per AllGather shard
NB = 8                      # dst blocks per core (128 rows each)
NBINS = NUM_CORES * NB
BIN_SZ = 128
F_IN, H1, H2, H3, F_OUT = 256, 2048, 2048, 1024, 768
DT = mybir.dt.float32
BF = mybir.dt.bfloat16
TANH = mybir.ActivationFunctionType.Tanh


# ----------------------------------------------------------------------------
# Host-side graph preprocessing
# ----------------------------------------------------------------------------

def _pack_idx(chunk_lists, tot_ch):
    """Pack per-chunk [128] source-row arrays into the SWDGE idx layout:
    [128, tot_ch * 8] int16, indices wrapped in 16 partitions and replicated
    8x across partition groups."""
    tab = np.zeros((128, tot_ch * 8), np.int16)
    ar = np.arange(128)
    for t, rows in enumerate(chunk_lists):
        tab[ar % 16, t * 8 + ar // 16] = rows.astype(np.int16)
    for rep in range(1, 8):
        tab[rep * 16:(rep + 1) * 16, :] = tab[:16, :]
    return tab


def _preprocess(edge_index):
    src = np.asarray(edge_index[0], dtype=np.int64)
    dst = np.asarray(edge_index[1], dtype=np.int64)

    deg = np.bincount(dst, minlength=N).astype(np.float64) + 1.0
    dinv = 1.0 / np.sqrt(deg)
    d_in = np.bincount(dst, minlength=N) + 1

    # greedy balanced partition of nodes into bins of 128, equal in-edge sums
    order = np.argsort(-d_in, kind="stable")
    bin_sum = np.zeros(NBINS, dtype=np.int64)
    bin_cnt = np.zeros(NBINS, dtype=np.int64)
    bin_nodes = [[] for _ in range(NBINS)]
    for node in order:
        avail = np.where(bin_cnt < BIN_SZ)[0]
        b = avail[np.argmin(bin_sum[avail])]
        bin_nodes[b].append(node)
        bin_sum[b] += d_in[node]
        bin_cnt[b] += 1

    target = int(np.ceil(d_in.sum() / NBINS))
    for _ in range(200):
        hi = int(np.argmax(bin_sum))
        if bin_sum[hi] <= target:
            break
        lo = int(np.argmin(bin_sum))
        need = bin_sum[hi] - target
        best = None
        for ai, a in enumerate(bin_nodes[hi]):
            for bi, b in enumerate(bin_nodes[lo]):
                diff = d_in[a] - d_in[b]
                if diff > 0:
                    score = abs(diff - need)
                    if best is None or score < best[0]:
                        best = (score, ai, bi)
        if best is None:
            break
        _, ai, bi = best
        a, b = bin_nodes[hi][ai], bin_nodes[lo][bi]
        bin_nodes[hi][ai], bin_nodes[lo][bi] = b, a
        bin_sum[hi] += d_in[b] - d_in[a]
        bin_sum[lo] += d_in[a] - d_in[b]

    CH1 = int(np.ceil(bin_sum.max() / 128))

    # Assign bins to slots so that no (dst block, src half) pair has more
    # than 5*128 edges: fullA/fullB each hold the union of every core's
    # half-A/half-B bins (slot%8 < 4 -> A), the L2/L3 gathers pad each
    # (block, half) to a multiple of 128 rows, and the assignment is free.
    src_bin = np.empty(N, np.int64)
    for i, bn in enumerate(bin_nodes):
        src_bin[np.array(bn, dtype=np.int64)] = i
    es = np.concatenate([src, np.arange(N, dtype=np.int64)])
    ed = np.concatenate([dst, np.arange(N, dtype=np.int64)])
    C = np.zeros((NBINS, NBINS), np.int64)
    np.add.at(C, (src_bin[ed], src_bin[es]), 1)

    # any 32/32 partition of bins into halves works (within-core slot
    # order is free): random restarts + swap hill-climb on max halfsum
    rng = np.random.default_rng(0)
    tot = C.sum(axis=1)

    def climb(memb, iters):
        MA = C[:, memb == 0].sum(axis=1)
        best = int(np.maximum(MA, tot - MA).max())
        for _ in range(iters):
            if best <= 640:
                break
            ia = rng.choice(np.where(memb == 0)[0])
            ib = rng.choice(np.where(memb == 1)[0])
            cand = MA + C[:, ib] - C[:, ia]
            mx = int(np.maximum(cand, tot - cand).max())
            if mx <= best:
                best = mx
                MA = cand
                memb[ia], memb[ib] = 1, 0
        return best, memb

    best_mx, best_memb = None, None
    for _ in range(300):
        memb = np.zeros(NBINS, np.int8)
        memb[rng.permutation(NBINS)[:NBINS // 2]] = 1
        MA = C[:, memb == 0].sum(axis=1)
        mx = int(np.maximum(MA, tot - MA).max())
        if best_mx is None or mx < best_mx:
            best_mx, best_memb = mx, memb.copy()
        if best_mx <= 600:
            break
    if best_mx > 600:
        best_mx, best_memb = climb(best_memb, 5000)

    # node-level refinement: swap equal-in-degree nodes across halves
    # (keeps every bin sum exact, so CH1 is unaffected) until no
    # (block, half) pair exceeds 5*128 edges
    node_bin = src_bin.copy()

    def half_counts(nb):
        m2 = np.zeros((NBINS, 2), np.int64)
        np.add.at(m2, (nb[ed], best_memb[nb[es]].astype(np.int64)), 1)
        return m2

    def m2_score(m2):
        mx = int(m2.max())
        return (mx, int((m2 > 640).sum()), int(m2[m2 > 640].sum()))

    m2 = half_counts(node_bin)
    cur = m2_score(m2)
    d_all = d_in  # includes the self loop
    for _ in range(400):
        if cur[0] <= 640:
            break
        b_star, h_star = np.unravel_index(np.argmax(m2), m2.shape)
        # sources (in half h_star) of edges into b_star
        cand = es[(node_bin[ed] == b_star)
                  & (best_memb[node_bin[es]] == h_star)]
        u = int(rng.choice(cand))
        other = np.where((best_memb[node_bin] != h_star)
                         & (d_all == d_all[u]))[0]
        if len(other) == 0:
            continue
        v = int(rng.choice(other))
        node_bin[u], node_bin[v] = node_bin[v], node_bin[u]
        m2_new = half_counts(node_bin)
        new = m2_score(m2_new)
        if new <= cur:
            cur, m2 = new, m2_new
        else:
            node_bin[u], node_bin[v] = node_bin[v], node_bin[u]

    bin_nodes = [list(np.where(node_bin == i)[0]) for i in range(NBINS)]
    a_bins = list(np.where(best_memb == 0)[0])
    b_bins = list(np.where(best_memb == 1)[0])
    slots = np.empty(NBINS, np.int64)
    for c in range(NUM_CORES):
        slots[c * 8:c * 8 + 4] = a_bins[c * 4:(c + 1) * 4]
        slots[c * 8 + 4:c * 8 + 8] = b_bins[c * 4:(c + 1) * 4]
    bin_nodes = [bin_nodes[s] for s in slots]
    bin_sum = np.array([int(d_in[bn].sum()) for bn in bin_nodes])

    perm = np.concatenate([np.array(bn, dtype=np.int64) for bn in bin_nodes])
    inv = np.empty(N, dtype=np.int64)
    inv[perm] = np.arange(N)

    all_src = np.concatenate([inv[src], np.arange(N, dtype=np.int64)])
    all_dst = np.concatenate([inv[dst], np.arange(N, dtype=np.int64)])
    all_w = np.concatenate([
        (dinv[src] * dinv[dst]).astype(np.float32),
        (dinv[perm] * dinv[perm]).astype(np.float32),
    ])

    bin_of = all_dst // BIN_SZ
    dst_local = all_dst % BIN_SZ
    src_half = (all_src % R) // HALF
    src_hrow = (all_src // R) * HALF + (all_src % HALF)

    # CHH: chunks per (block, half) for the L2/L3 gathers
    CHH = 0
    for b in range(NBINS):
        for h in range(2):
            n = int(np.sum((bin_of == b) & (src_half == h)))
            CHH = max(CHH, (n + 127) // 128)

    idx1_tabs, s1_tabs, idx2_tabs, s2_tabs = [], [], [], []
    for c in range(NUM_CORES):
        chunks1, chunks2 = [], []
        s1 = np.zeros((128, NB * CH1, 128), np.float32)
        s2 = np.zeros((128, NB * 2 * CHH, 128), np.float32)
        for blk in range(NB):
            sel = np.where(bin_of == c * NB + blk)[0]
            # L1 table: all edges of the block, sources are x_perm rows
            e_src = np.zeros(CH1 * 128, np.int64)
            e_dst = np.zeros(CH1 * 128, np.int64)
            e_w = np.zeros(CH1 * 128, np.float32)
            e_src[:len(sel)] = all_src[sel]
            e_dst[:len(sel)] = dst_local[sel]
            e_w[:len(sel)] = all_w[sel]
            for ci in range(CH1):
                sl = slice(ci * 128, (ci + 1) * 128)
                chunks1.append(e_src[sl])
                s1[np.arange(128), blk * CH1 + ci, e_dst[sl]] = e_w[sl]
            # L2/L3 table: edges split by source half, row ids in half tensor
            for h in range(2):
                selh = sel[src_half[sel] == h]
                e_src = np.zeros(CHH * 128, np.int64)
                e_dst = np.zeros(CHH * 128, np.int64)
                e_w = np.zeros(CHH * 128, np.float32)
                e_src[:len(selh)] = src_hrow[selh]
                e_dst[:len(selh)] = dst_local[selh]
                e_w[:len(selh)] = all_w[selh]
                for ci in range(CHH):
                    sl = slice(ci * 128, (ci + 1) * 128)
                    chunks2.append(e_src[sl])
                    t = blk * 2 * CHH + h * CHH + ci
                    s2[np.arange(128), t, e_dst[sl]] = e_w[sl]
        idx1_tabs.append(_pack_idx(chunks1, NB * CH1))
        s1_tabs.append(s1)
        idx2_tabs.append(_pack_idx(chunks2, NB * 2 * CHH))
        s2_tabs.append(s2)

    return perm, CH1, CHH, idx1_tabs, s1_tabs, idx2_tabs, s2_tabs


# ----------------------------------------------------------------------------
# Device program
# ----------------------------------------------------------------------------

def _build_program(CH1, CHH):
    T1 = NB * CH1
    T2 = NB * 2 * CHH
    nc = bacc.Bacc("TRN2", target_bir_lowering=False, debug=False,
                   num_devices=NUM_CORES)
    core_ids = list(range(NUM_CORES))

    x_perm = nc.dram_tensor("x_perm", [N, F_IN], BF, kind="ExternalInput")
    idx1_in = nc.dram_tensor("idx1_in", [128, T1 * 8], mybir.dt.int16,
                             kind="ExternalInput")
    s1_in = nc.dram_tensor("s1_in", [128, T1, 128], BF, kind="ExternalInput")
    idx2_in = nc.dram_tensor("idx2_in", [128, T2 * 8], mybir.dt.int16,
                             kind="ExternalInput")
    s2_in = nc.dram_tensor("s2_in", [128, T2, 128], BF, kind="ExternalInput")
    ident_in = nc.dram_tensor("ident", [128, 128], BF, kind="ExternalInput")
    # weights pre-tiled on host to [128, K/128, F] layout
    w1t_in = nc.dram_tensor("w1t", [128, F_IN // 128, H1], BF, kind="ExternalInput")
    w2t_in = nc.dram_tensor("w2t", [128, H1 // 128, H2], BF, kind="ExternalInput")
    w3t_in = nc.dram_tensor("w3t", [128, H2 // 128, H3], BF, kind="ExternalInput")
    wlt_in = nc.dram_tensor("wlt", [128, H3 // 128, F_OUT], BF, kind="ExternalInput")
    b1_in = nc.dram_tensor("b1pp", [128, H1 // 128], DT, kind="ExternalInput")
    b2_in = nc.dram_tensor("b2pp", [128, H2 // 128], DT, kind="ExternalInput")
    b3_in = nc.dram_tensor("b3pp", [128, H3 // 128], DT, kind="ExternalInput")
    bl_in = nc.dram_tensor("blb", [128, F_OUT], DT, kind="ExternalInput")

    out = nc.dram_tensor("out", [R, F_OUT], DT, kind="ExternalOutput")

    xw2_locs = [nc.dram_tensor(f"xw2_loc{h}", [HALF, H2], BF) for h in range(2)]
    xw2_fulls = [nc.dram_tensor(f"xw2_full{h}", [NUM_CORES * HALF, H2], BF,
                                addr_space="Shared") for h in range(2)]
    xw3_locs = [nc.dram_tensor(f"xw3_loc{h}", [HALF, H3], BF) for h in range(2)]
    xw3_fulls = [nc.dram_tensor(f"xw3_full{h}", [NUM_CORES * HALF, H3], BF,
                                addr_space="Shared") for h in range(2)]

    uid = [0]

    def pname(base):
        uid[0] += 1
        return f"{base}{uid[0]}"

    with tile.TileContext(nc) as tc:
        with tc.tile_pool(name="const", bufs=1) as cpool:
            nc.gpsimd.load_library(mlp)
            idx1_sb = cpool.tile([128, T1 * 8], mybir.dt.int16, tag="idx1")
            nc.sync.dma_start(idx1_sb[:], idx1_in[:])
            s1_sb = cpool.tile([128, T1, 128], BF, tag="s1")
            nc.sync.dma_start(s1_sb[:], s1_in[:])
            idx2_sb = cpool.tile([128, T2 * 8], mybir.dt.int16, tag="idx2")
            nc.sync.dma_start(idx2_sb[:], idx2_in[:])
            s2_sb = cpool.tile([128, T2, 128], BF, tag="s2")
            nc.sync.dma_start(s2_sb[:], s2_in[:])
            id_sb = cpool.tile([128, 128], BF, tag="ident")
            nc.sync.dma_start(id_sb[:], ident_in[:])
            b1_sb = cpool.tile([128, H1 // 128], DT, tag="b1")
            nc.sync.dma_start(b1_sb[:], b1_in[:])
            b2_sb = cpool.tile([128, H2 // 128], DT, tag="b2")
            nc.sync.dma_start(b2_sb[:], b2_in[:])
            b3_sb = cpool.tile([128, H3 // 128], DT, tag="b3")
            nc.sync.dma_start(b3_sb[:], b3_in[:])
            bl_sb = cpool.tile([128, F_OUT], DT, tag="bl")
            nc.sync.dma_start(bl_sb[:], bl_in[:])

            def phase_L1(h1t):
                """gather x, aggregate node-major, transpose, transform+tanh."""
                with (
                    tc.tile_pool(name=pname("l1a"), bufs=1) as l1a_pool,
                    tc.tile_pool(name=pname("l1ps"), bufs=1, space="PSUM") as l1ps,
                ):
                    w1t_sb = l1a_pool.tile([128, F_IN // 128, H1], BF, tag="w1t")
                    nc.sync.dma_start(w1t_sb[:], w1t_in[:])
                    agg1t = l1a_pool.tile([128, F_IN // 128, R], BF, tag="agg1t")
                    for blk in range(NB):
                        g = l1a_pool.tile([128, CH1, F_IN], BF, tag="g1", bufs=2)
                        nc.gpsimd.dma_gather(
                            g[:], x_perm[:],
                            idx1_sb[:, blk * CH1 * 8:(blk + 1) * CH1 * 8],
                            CH1 * 128, CH1 * 128, F_IN, single_packet=False)
                        ps = l1ps.tile([128, F_IN], DT, tag="agg", bufs=2)
                        for c in range(CH1):
                            nc.tensor.matmul(
                                ps[:, :], s1_sb[:, blk * CH1 + c, :], g[:, c, :],
                                start=(c == 0), stop=(c == CH1 - 1))
                        a_nm = l1a_pool.tile([128, F_IN], BF, tag="anm", bufs=2)
                        nc.vector.tensor_copy(a_nm[:], ps[:])
                        for f in range(F_IN // 128):
                            pt = l1ps.tile([128, 128], BF, tag="pt", bufs=2)
                            nc.tensor.transpose(
                                pt[:], a_nm[:, f * 128:(f + 1) * 128], id_sb[:])
                            nc.vector.tensor_copy(
                                agg1t[:, f, blk * 128:(blk + 1) * 128], pt[:])
                    for m in range(H1 // 128):
                        ps = l1ps.tile([128, R], DT, tag="xw", bufs=2)
                        for k in range(F_IN // 128):
                            for n in range(0, R, 512):
                                nc.tensor.matmul(
                                    ps[:, n:n + 512],
                                    w1t_sb[:, k, m * 128:(m + 1) * 128],
                                    agg1t[:, k, n:n + 512],
                                    start=(k == 0), stop=(k == F_IN // 128 - 1))
                        nc.scalar.activation(
                            h1t[:, m, :], ps[:], TANH, bias=b1_sb[:, m:m + 1])

            def transform(ht, KD, FD, wt_in, locs, fulls):
                """locs[h] = rows [h*512, h*512+512) of (ht rows) @ W^T;
                AllGather each row-half as soon as it is written so the
                collective overlaps with the other half's matmuls."""
                HK = KD // 128
                with (
                    tc.tile_pool(name=pname("tr"), bufs=1) as tpool,
                    tc.tile_pool(name=pname("trps"), bufs=1, space="PSUM") as tps,
                ):
                    wt_sb = tpool.tile([128, HK, FD], BF, tag="wt")
                    nc.sync.dma_start(wt_sb[:], wt_in[:])
                    for h in range(2):
                        for r4 in range(HALF // 128):
                            r = h * (HALF // 128) + r4
                            ps = tps.tile([128, FD], DT, tag="xw", bufs=2)
                            for k in range(HK):
                                for n0 in range(0, FD, 512):
                                    n1 = min(n0 + 512, FD)
                                    nc.tensor.matmul(
                                        ps[:, n0:n1],
                                        ht[:, k, r * 128:(r + 1) * 128],
                                        wt_sb[:, k, n0:n1],
                                        start=(k == 0), stop=(k == HK - 1))
                            o = tpool.tile([128, FD], BF, tag="o", bufs=3)
                            nc.vector.tensor_copy(o[:], ps[:])
                            nc.sync.dma_start(
                                locs[h][r4 * 128:(r4 + 1) * 128, :], o[:])
                        nc.gpsimd.collective_compute(
                            "AllGather", mybir.AluOpType.bypass,
                            replica_groups=[core_ids],
                            ins=[locs[h][:]], outs=[fulls[h][:]])

            def aggregate(fulls, FD, ht, b_sb):
                """gather full-width rows by edge sources (one DMA row per
                edge), node-major reduce via S matmuls, transpose + tanh(.+b)
                into feature-major ht.  All half-A work first (gates only on
                AllGather A); partials park in SBUF and fold in via the
                half-B copy-out."""
                with (
                    tc.tile_pool(name=pname("ag"), bufs=1) as apool,
                    tc.tile_pool(name=pname("agps"), bufs=1, space="PSUM") as aps,
                ):
                    a_part = apool.tile([128, NB, FD], DT, tag="apart")
                    for h in range(2):
                        for blk in range(NB):
                            t0 = blk * 2 * CHH + h * CHH
                            g = apool.tile([128, CHH, FD], BF, tag="g", bufs=2)
                            nc.gpsimd.dma_gather(
                                g[:], fulls[h][:],
                                idx2_sb[:, t0 * 8:(t0 + CHH) * 8],
                                CHH * 128, CHH * 128, FD, single_packet=False)
                            ps = aps.tile([128, FD], DT, tag="agg", bufs=1)
                            for c in range(CHH):
                                for n0 in range(0, FD, 512):
                                    nc.tensor.matmul(
                                        ps[:, n0:n0 + 512],
                                        s2_sb[:, t0 + c, :],
                                        g[:, c, n0:n0 + 512],
                                        start=(c == 0), stop=(c == CHH - 1))
                            if h == 0:
                                nc.vector.tensor_copy(a_part[:, blk, :], ps[:])
                            else:
                                a_nm = apool.tile([128, FD], BF, tag="anm",
                                                  bufs=2)
                                nc.vector.tensor_tensor(
                                    out=a_nm[:], in0=ps[:],
                                    in1=a_part[:, blk, :],
                                    op=mybir.AluOpType.add)
                                for f in range(FD // 128):
                                    pt = aps.tile([128, 128], BF, tag="pt",
                                                  bufs=4)
                                    nc.tensor.transpose(
                                        pt[:], a_nm[:, f * 128:(f + 1) * 128],
                                        id_sb[:])
                                    nc.scalar.activation(
                                        ht[:, f, blk * 128:(blk + 1) * 128],
                                        pt[:], TANH, bias=b_sb[:, f:f + 1])

            def phase_FIN(h3t):
                with (
                    tc.tile_pool(name=pname("fin"), bufs=1) as fpool,
                    tc.tile_pool(name=pname("finps"), bufs=1, space="PSUM") as fps,
                ):
                    wlt_sb = fpool.tile([128, H3 // 128, F_OUT], BF, tag="wlt")
                    nc.sync.dma_start(wlt_sb[:], wlt_in[:])
                    for r in range(NB):
                        ps = fps.tile([128, F_OUT], DT, tag="xw", bufs=2)
                        for k in range(H3 // 128):
                            for n0 in range(0, F_OUT, 512):
                                n1 = min(n0 + 512, F_OUT)
                                nc.tensor.matmul(
                                    ps[:, n0:n1],
                                    h3t[:, k, r * 128:(r + 1) * 128],
                                    wlt_sb[:, k, n0:n1],
                                    start=(k == 0), stop=(k == H3 // 128 - 1))
                        o = fpool.tile([128, F_OUT], DT, tag="o", bufs=3)
                        nc.vector.tensor_tensor(
                            out=o[:], in0=ps[:], in1=bl_sb[:],
                            op=mybir.AluOpType.add)
                        nc.sync.dma_start(out[r * 128:(r + 1) * 128, :], o[:])

            with tc.tile_pool(name=pname("h1t"), bufs=1) as h1t_pool:
                h1t = h1t_pool.tile([128, H1 // 128, R], BF, tag="h1t")
                phase_L1(h1t)
                transform(h1t, H1, H2, w2t_in, xw2_locs, xw2_fulls)
            with tc.tile_pool(name=pname("h2t"), bufs=1) as h2t_pool:
                h2t = h2t_pool.tile([128, H2 // 128, R], BF, tag="h2t")
                aggregate(xw2_fulls, H2, h2t, b2_sb)
                transform(h2t, H2, H3, w3t_in, xw3_locs, xw3_fulls)
            with tc.tile_pool(name=pname("h3t"), bufs=1) as h3t_pool:
                h3t = h3t_pool.tile([128, H3 // 128, R], BF, tag="h3t")
                aggregate(xw3_fulls, H3, h3t, b3_sb)
                phase_FIN(h3t)

    nc.compile()
    return nc


# ----------------------------------------------------------------------------
# Entry point
# ----------------------------------------------------------------------------

def _make_in_maps(inputs, perm, pre):
    import ml_dtypes
    bf = ml_dtypes.bfloat16
    _, _, _, idx1_tabs, s1_tabs, idx2_tabs, s2_tabs = pre

    def tile_w(w):  # [K, F] -> [128, K/128, F]
        k, f = w.shape
        return np.ascontiguousarray(
            w.reshape(k // 128, 128, f).transpose(1, 0, 2)).astype(bf)

    x_perm = np.ascontiguousarray(
        np.asarray(inputs["x"], np.float32)[perm]).astype(bf)
    w1t = tile_w(np.ascontiguousarray(np.asarray(inputs["W1"], np.float32).T))
    w2t = tile_w(np.ascontiguousarray(np.asarray(inputs["W2"], np.float32).T))
    w3t = tile_w(np.ascontiguousarray(np.asarray(inputs["W3"], np.float32).T))
    wlt = tile_w(np.ascontiguousarray(np.asarray(inputs["Wl"], np.float32).T))
    b1pp = np.ascontiguousarray(
        np.asarray(inputs["b1"], np.float32).reshape(-1, 128).T)
    b2pp = np.ascontiguousarray(
        np.asarray(inputs["b2"], np.float32).reshape(-1, 128).T)
    b3pp = np.ascontiguousarray(
        np.asarray(inputs["b3"], np.float32).reshape(-1, 128).T)
    blb = np.ascontiguousarray(
        np.broadcast_to(np.asarray(inputs["bl"], np.float32), (128, F_OUT)))
    ident = np.eye(128, dtype=bf)

    in_maps = []
    for c in range(NUM_CORES):
        in_maps.append({
            "x_perm": x_perm,
            "idx1_in": idx1_tabs[c], "s1_in": s1_tabs[c].astype(bf),
            "idx2_in": idx2_tabs[c], "s2_in": s2_tabs[c].astype(bf),
            "ident": ident,
            "w1t": w1t, "w2t": w2t, "w3t": w3t, "wlt": wlt,
            "b1pp": b1pp, "b2pp": b2pp, "b3pp": b3pp, "blb": blb,
        })
    return in_maps


def _run(inputs, trace=False):
    pre = _preprocess(np.asarray(inputs["edge_index"]))
    perm, CH1, CHH = pre[0], pre[1], pre[2]
    nc = _build_program(CH1, CHH)
    in_maps = _make_in_maps(inputs, perm, pre)
    res = run_bass_kernel_spmd(nc, in_maps, list(range(NUM_CORES)), trace=trace)
    out_perm = np.concatenate([res.results[c]["out"] for c in range(NUM_CORES)], 0)
    out = np.empty_like(out_perm)
    out[perm] = out_perm
    return out, res


def kernel(**inputs):
    out, _ = _run(inputs, trace=False)
    return out


# revision 31
# speedup vs baseline: 1.0619x; 1.0008x over previous
"""3-layer GCN + linear head on 8 Trainium2 NeuronCores.

Sharding: nodes are partitioned across the 8 cores (graph parallel), after a
host-side balanced permutation that gives every 128-node block exactly the
same number of incoming edges (including self loops).  All message traffic is
bf16 (tolerance is 2e-2); PSUM accumulation is fp32.

Per layer (2, 3) each core:
  - transforms its local rows 0-511 (dense matmul, weights replicated),
    AllGathers them into fullA [4096, FD] while transforming rows 512-1023,
    which AllGather into fullB,
  - gathers edge-source rows (full-width, 1 DMA descriptor per edge) with
    SWDGE dma_gather and reduces them into destination rows with TensorE
    matmuls against host-built per-chunk selection matrices S (which carry
    the GCN edge normalization weights).  All A-half gathers for the 8 dst
    blocks run first (they only gate on AllGather #1, overlapping AllGather
    #2); partial sums park in SBUF and are folded in during the B half.
Layer 1 aggregates x first (256-dim messages) and transforms after.  The
final linear head has no aggregation.
"""
import sys
if "/opt/trn_rl_repo" not in sys.path:
    sys.path.insert(0, "/opt/trn_rl_repo")

import numpy as np

import concourse.bass as bass
import concourse.mybir as mybir
import concourse.tile as tile
from concourse import bacc
from concourse.bass_utils import run_bass_kernel_spmd
from concourse.library_config import mlp

N = 8192
NUM_CORES = 8
R = N // NUM_CORES          # rows per core
HALF = R // 2               # rows per AllGather shard
NB = 8                      # dst blocks per core (128 rows each)
NBINS = NUM_CORES * NB
BIN_SZ = 128
F_IN, H1, H2, H3, F_OUT = 256, 2048, 2048, 1024, 768
DT = mybir.dt.float32
BF = mybir.dt.bfloat16
TANH = mybir.ActivationFunctionType.Tanh


# ----------------------------------------------------------------------------
# Host-side graph preprocessing
# ----------------------------------------------------------------------------

def _pack_idx(chunk_lists, tot_ch):
    """Pack per-chunk [128] source-row arrays into the SWDGE idx layout:
    [128, tot_ch * 8] int16, indices wrapped in 16 partitions and replicated
    8x across partition groups."""
    tab = np.zeros((128, tot_ch * 8), np.int16)
    ar = np.arange(128)
    for t, rows in enumerate(chunk_lists):
        tab[ar % 16, t * 8 + ar // 16] = rows.astype(np.int16)
    for rep in range(1, 8):
        tab[rep * 16:(rep + 1) * 16, :] = tab[:16, :]
    return tab


def _preprocess(edge_index):
    src = np.asarray(edge_index[0], dtype=np.int64)
    dst = np.asarray(edge_index[1], dtype=np.int64)

    deg = np.bincount(dst, minlength=N).astype(np.float64) + 1.0
    dinv = 1.0 / np.sqrt(deg)
    d_in = np.bincount(dst, minlength=N) + 1

    # greedy balanced partition of nodes into bins of 128, equal in-edge sums
    order = np.argsort(-d_in, kind="stable")
    bin_sum = np.zeros(NBINS, dtype=np.int64)
    bin_cnt = np.zeros(NBINS, dtype=np.int64)
    bin_nodes = [[] for _ in range(NBINS)]
    for node in order:
        avail = np.where(bin_cnt < BIN_SZ)[0]
        b = avail[np.argmin(bin_sum[avail])]
        bin_nodes[b].append(node)
        bin_sum[b] += d_in[node]
        bin_cnt[b] += 1

    target = int(np.ceil(d_in.sum() / NBINS))
    for _ in range(200):
        hi = int(np.argmax(bin_sum))
        if bin_sum[hi] <= target:
            break
        lo = int(np.argmin(bin_sum))
        need = bin_sum[hi] - target
        best = None
        for ai, a in enumerate(bin_nodes[hi]):
            for bi, b in enumerate(bin_nodes[lo]):
                diff = d_in[a] - d_in[b]
                if diff > 0:
                    score = abs(diff - need)
                    if best is None or score < best[0]:
                        best = (score, ai, bi)
        if best is None:
            break
        _, ai, bi = best
        a, b = bin_nodes[hi][ai], bin_nodes[lo][bi]
        bin_nodes[hi][ai], bin_nodes[lo][bi] = b, a
        bin_sum[hi] += d_in[b] - d_in[a]
        bin_sum[lo] += d_in[a] - d_in[b]

    CH1 = int(np.ceil(bin_sum.max() / 128))

    # Assign bins to slots so that no (dst block, src half) pair has more
    # than 5*128 edges: fullA/fullB each hold the union of every core's
    # half-A/half-B bins (slot%8 < 4 -> A), the L2/L3 gathers pad each
    # (block, half) to a multiple of 128 rows, and the assignment is free.
    src_bin = np.empty(N, np.int64)
    for i, bn in enumerate(bin_nodes):
        src_bin[np.array(bn, dtype=np.int64)] = i
    es = np.concatenate([src, np.arange(N, dtype=np.int64)])
    ed = np.concatenate([dst, np.arange(N, dtype=np.int64)])
    C = np.zeros((NBINS, NBINS), np.int64)
    np.add.at(C, (src_bin[ed], src_bin[es]), 1)

    # any 32/32 partition of bins into halves works (within-core slot
    # order is free): random restarts + swap hill-climb on max halfsum
    rng = np.random.default_rng(0)
    tot = C.sum(axis=1)

    def climb(memb, iters):
        MA = C[:, memb == 0].sum(axis=1)
        best = int(np.maximum(MA, tot - MA).max())
        for _ in range(iters):
            if best <= 640:
                break
            ia = rng.choice(np.where(memb == 0)[0])
            ib = rng.choice(np.where(memb == 1)[0])
            cand = MA + C[:, ib] - C[:, ia]
            mx = int(np.maximum(cand, tot - cand).max())
            if mx <= best:
                best = mx
                MA = cand
                memb[ia], memb[ib] = 1, 0
        return best, memb

    best_mx, best_memb = None, None
    for _ in range(300):
        memb = np.zeros(NBINS, np.int8)
        memb[rng.permutation(NBINS)[:NBINS // 2]] = 1
        MA = C[:, memb == 0].sum(axis=1)
        mx = int(np.maximum(MA, tot - MA).max())
        if best_mx is None or mx < best_mx:
            best_mx, best_memb = mx, memb.copy()
        if best_mx <= 600:
            break
    if best_mx > 600:
        best_mx, best_memb = climb(best_memb, 5000)

    # node-level refinement: swap equal-in-degree nodes across halves
    # (keeps every bin sum exact, so CH1 is unaffected) until no
    # (block, half) pair exceeds 5*128 edges
    node_bin = src_bin.copy()

    def half_counts(nb):
        m2 = np.zeros((NBINS, 2), np.int64)
        np.add.at(m2, (nb[ed], best_memb[nb[es]].astype(np.int64)), 1)
        return m2

    def m2_score(m2):
        mx = int(m2.max())
        return (mx, int((m2 > 640).sum()), int(m2[m2 > 640].sum()))

    m2 = half_counts(node_bin)
    cur = m2_score(m2)
    d_all = d_in  # includes the self loop
    for _ in range(400):
        if cur[0] <= 640:
            break
        b_star, h_star = np.unravel_index(np.argmax(m2), m2.shape)
        # sources (in half h_star) of edges into b_star
        cand = es[(node_bin[ed] == b_star)
                  & (best_memb[node_bin[es]] == h_star)]
        u = int(rng.choice(cand))
        other = np.where((best_memb[node_bin] != h_star)
                         & (d_all == d_all[u]))[0]
        if len(other) == 0:
            continue
        v = int(rng.choice(other))
        node_bin[u], node_bin[v] = node_bin[v], node_bin[u]
        m2_new = half_counts(node_bin)
        new = m2_score(m2_new)
        if new <= cur:
            cur, m2 = new, m2_new
        else:
            node_bin[u], node_bin[v] = node_bin[v], node_bin[u]

    bin_nodes = [list(np.where(node_bin == i)[0]) for i in range(NBINS)]
    a_bins = list(np.where(best_memb == 0)[0])
    b_bins = list(np.where(best_memb == 1)[0])
    slots = np.empty(NBINS, np.int64)
    for c in range(NUM_CORES):
        slots[c * 8:c * 8 + 4] = a_bins[c * 4:(c + 1) * 4]
        slots[c * 8 + 4:c * 8 + 8] = b_bins[c * 4:(c + 1) * 4]
    bin_nodes = [bin_nodes[s] for s in slots]
    bin_sum = np.array([int(d_in[bn].sum()) for bn in bin_nodes])

    perm = np.concatenate([np.array(bn, dtype=np.int64) for bn in bin_nodes])
    inv = np.empty(N, dtype=np.int64)
    inv[perm] = np.arange(N)

    all_src = np.concatenate([inv[src], np.arange(N, dtype=np.int64)])
    all_dst = np.concatenate([inv[dst], np.arange(N, dtype=np.int64)])
    all_w = np.concatenate([
        (dinv[src] * dinv[dst]).astype(np.float32),
        (dinv[perm] * dinv[perm]).astype(np.float32),
    ])

    bin_of = all_dst // BIN_SZ
    dst_local = all_dst % BIN_SZ
    src_half = (all_src % R) // HALF
    src_hrow = (all_src // R) * HALF + (all_src % HALF)

    # CHH: chunks per (block, half) for the L2/L3 gathers
    CHH = 0
    for b in range(NBINS):
        for h in range(2):
            n = int(np.sum((bin_of == b) & (src_half == h)))
            CHH = max(CHH, (n + 127) // 128)

    idx1_tabs, s1_tabs, idx2_tabs, s2_tabs = [], [], [], []
    for c in range(NUM_CORES):
        chunks1, chunks2 = [], []
        s1 = np.zeros((128, NB * CH1, 128), np.float32)
        s2 = np.zeros((128, NB * 2 * CHH, 128), np.float32)
        for blk in range(NB):
            sel = np.where(bin_of == c * NB + blk)[0]
            # L1 table: all edges of the block, sources are x_perm rows
            e_src = np.zeros(CH1 * 128, np.int64)
            e_dst = np.zeros(CH1 * 128, np.int64)
            e_w = np.zeros(CH1 * 128, np.float32)
            e_src[:len(sel)] = all_src[sel]
            e_dst[:len(sel)] = dst_local[sel]
            e_w[:len(sel)] = all_w[sel]
            for ci in range(CH1):
                sl = slice(ci * 128, (ci + 1) * 128)
                chunks1.append(e_src[sl])
                s1[np.arange(128), blk * CH1 + ci, e_dst[sl]] = e_w[sl]
            # L2/L3 table: edges split by source half, row ids in half tensor
            for h in range(2):
                selh = sel[src_half[sel] == h]
                e_src = np.zeros(CHH * 128, np.int64)
                e_dst = np.zeros(CHH * 128, np.int64)
                e_w = np.zeros(CHH * 128, np.float32)
                e_src[:len(selh)] = src_hrow[selh]
                e_dst[:len(selh)] = dst_local[selh]
                e_w[:len(selh)] = all_w[selh]
                for ci in range(CHH):
                    sl = slice(ci * 128, (ci + 1) * 128)
                    chunks2.append(e_src[sl])
                    t = blk * 2 * CHH + h * CHH + ci
                    s2[np.arange(128), t, e_dst[sl]] = e_w[sl]
        idx1_tabs.append(_pack_idx(chunks1, NB * CH1))
        s1_tabs.append(s1)
        idx2_tabs.append(_pack_idx(chunks2, NB * 2 * CHH))
        s2_tabs.append(s2)

    return perm, CH1, CHH, idx1_tabs, s1_tabs, idx2_tabs, s2_tabs


# ----------------------------------------------------------------------------
# Device program
# ----------------------------------------------------------------------------

def _build_program(CH1, CHH, zero_b23=True):
    T1 = NB * CH1
    T2 = NB * 2 * CHH
    nc = bacc.Bacc("TRN2", target_bir_lowering=False, debug=False,
                   num_devices=NUM_CORES)
    core_ids = list(range(NUM_CORES))

    x_perm = nc.dram_tensor("x_perm", [N, F_IN], BF, kind="ExternalInput")
    idx1_in = nc.dram_tensor("idx1_in", [128, T1 * 8], mybir.dt.int16,
                             kind="ExternalInput")
    s1_in = nc.dram_tensor("s1_in", [128, T1, 128], BF, kind="ExternalInput")
    idx2_in = nc.dram_tensor("idx2_in", [128, T2 * 8], mybir.dt.int16,
                             kind="ExternalInput")
    s2_in = nc.dram_tensor("s2_in", [128, T2, 128], BF, kind="ExternalInput")
    ident_in = nc.dram_tensor("ident", [128, 128], BF, kind="ExternalInput")
    # weights pre-tiled on host to [128, K/128, F] layout
    w1t_in = nc.dram_tensor("w1t", [128, F_IN // 128, H1], BF, kind="ExternalInput")
    w2t_in = nc.dram_tensor("w2t", [128, H1 // 128, H2], BF, kind="ExternalInput")
    w3t_in = nc.dram_tensor("w3t", [128, H2 // 128, H3], BF, kind="ExternalInput")
    wlt_in = nc.dram_tensor("wlt", [128, H3 // 128, F_OUT], BF, kind="ExternalInput")
    b1_in = nc.dram_tensor("b1pp", [128, H1 // 128], DT, kind="ExternalInput")
    if not zero_b23:
        b2_in = nc.dram_tensor("b2row", [128, H2], DT, kind="ExternalInput")
        b3_in = nc.dram_tensor("b3row", [128, H3], DT, kind="ExternalInput")
    bl_in = nc.dram_tensor("blb", [128, F_OUT], DT, kind="ExternalInput")

    out = nc.dram_tensor("out", [R, F_OUT], DT, kind="ExternalOutput")

    xw2_locs = [nc.dram_tensor(f"xw2_loc{h}", [HALF, H2], BF) for h in range(2)]
    xw2_fulls = [nc.dram_tensor(f"xw2_full{h}", [NUM_CORES * HALF, H2], BF,
                                addr_space="Shared") for h in range(2)]
    xw3_locs = [nc.dram_tensor(f"xw3_loc{h}", [HALF, H3], BF) for h in range(2)]
    xw3_fulls = [nc.dram_tensor(f"xw3_full{h}", [NUM_CORES * HALF, H3], BF,
                                addr_space="Shared") for h in range(2)]

    uid = [0]

    def pname(base):
        uid[0] += 1
        return f"{base}{uid[0]}"

    with tile.TileContext(nc) as tc:
        with tc.tile_pool(name="const", bufs=1) as cpool:
            nc.gpsimd.load_library(mlp)
            idx2_sb = cpool.tile([128, T2 * 8], mybir.dt.int16, tag="idx2")
            nc.sync.dma_start(idx2_sb[:], idx2_in[:])
            s2_sb = cpool.tile([128, T2, 128], BF, tag="s2")
            nc.sync.dma_start(s2_sb[:], s2_in[:])
            id_sb = cpool.tile([128, 128], BF, tag="ident")
            nc.sync.dma_start(id_sb[:], ident_in[:])
            b1_sb = cpool.tile([128, H1 // 128], DT, tag="b1")
            nc.sync.dma_start(b1_sb[:], b1_in[:])
            if zero_b23:
                b2_sb = b3_sb = None
            else:
                b2_sb = cpool.tile([128, H2], DT, tag="b2")
                nc.sync.dma_start(b2_sb[:], b2_in[:])
                b3_sb = cpool.tile([128, H3], DT, tag="b3")
                nc.sync.dma_start(b3_sb[:], b3_in[:])
            bl_sb = cpool.tile([128, F_OUT], DT, tag="bl")
            nc.sync.dma_start(bl_sb[:], bl_in[:])

            def phase_L1(h1t):
                """gather x, aggregate node-major, transpose, transform+tanh."""
                with (
                    tc.tile_pool(name=pname("l1a"), bufs=1) as l1a_pool,
                    tc.tile_pool(name=pname("l1ps"), bufs=1, space="PSUM") as l1ps,
                ):
                    idx1_sb = l1a_pool.tile([128, T1 * 8], mybir.dt.int16,
                                            tag="idx1")
                    nc.sync.dma_start(idx1_sb[:], idx1_in[:])
                    s1_sb = l1a_pool.tile([128, T1, 128], BF, tag="s1")
                    nc.sync.dma_start(s1_sb[:], s1_in[:])
                    w1t_sb = l1a_pool.tile([128, F_IN // 128, H1], BF, tag="w1t")
                    nc.sync.dma_start(w1t_sb[:], w1t_in[:])
                    agg1t = l1a_pool.tile([128, F_IN // 128, R], BF, tag="agg1t")
                    for blk in range(NB):
                        g = l1a_pool.tile([128, CH1, F_IN], BF, tag="g1", bufs=2)
                        nc.gpsimd.dma_gather(
                            g[:], x_perm[:],
                            idx1_sb[:, blk * CH1 * 8:(blk + 1) * CH1 * 8],
                            CH1 * 128, CH1 * 128, F_IN, single_packet=False)
                        ps = l1ps.tile([128, F_IN], DT, tag="agg", bufs=2)
                        for c in range(CH1):
                            nc.tensor.matmul(
                                ps[:, :], s1_sb[:, blk * CH1 + c, :], g[:, c, :],
                                start=(c == 0), stop=(c == CH1 - 1))
                        a_nm = l1a_pool.tile([128, F_IN], BF, tag="anm", bufs=2)
                        nc.vector.tensor_copy(a_nm[:], ps[:])
                        for f in range(F_IN // 128):
                            pt = l1ps.tile([128, 128], BF, tag="pt", bufs=2)
                            nc.tensor.transpose(
                                pt[:], a_nm[:, f * 128:(f + 1) * 128], id_sb[:])
                            nc.vector.tensor_copy(
                                agg1t[:, f, blk * 128:(blk + 1) * 128], pt[:])
                    for m in range(H1 // 128):
                        ps = l1ps.tile([128, R], DT, tag="xw", bufs=2)
                        for k in range(F_IN // 128):
                            for n in range(0, R, 512):
                                nc.tensor.matmul(
                                    ps[:, n:n + 512],
                                    w1t_sb[:, k, m * 128:(m + 1) * 128],
                                    agg1t[:, k, n:n + 512],
                                    start=(k == 0), stop=(k == F_IN // 128 - 1))
                        nc.scalar.activation(
                            h1t[:, m, :], ps[:], TANH, bias=b1_sb[:, m:m + 1])

            def transform(ht, KD, FD, wt_sb, locs, fulls):
                """locs[h] = rows [h*512, h*512+512) of (ht rows) @ W^T;
                AllGather each row-half as soon as it is written so the
                collective overlaps with the other half's matmuls."""
                HK = KD // 128
                with (
                    tc.tile_pool(name=pname("tr"), bufs=1) as tpool,
                    tc.tile_pool(name=pname("trps"), bufs=1, space="PSUM") as tps,
                ):
                    for h in range(2):
                        for r4 in range(HALF // 128):
                            r = h * (HALF // 128) + r4
                            ps = tps.tile([128, FD], DT, tag="xw", bufs=2)
                            for k in range(HK):
                                for n0 in range(0, FD, 512):
                                    n1 = min(n0 + 512, FD)
                                    nc.tensor.matmul(
                                        ps[:, n0:n1],
                                        ht[:, k, r * 128:(r + 1) * 128],
                                        wt_sb[:, k, n0:n1],
                                        start=(k == 0), stop=(k == HK - 1))
                            o = tpool.tile([128, FD], BF, tag="o", bufs=3)
                            nc.vector.tensor_copy(o[:], ps[:])
                            nc.sync.dma_start(
                                locs[h][r4 * 128:(r4 + 1) * 128, :], o[:])
                        nc.gpsimd.collective_compute(
                            "AllGather", mybir.AluOpType.bypass,
                            replica_groups=[core_ids],
                            ins=[locs[h][:]], outs=[fulls[h][:]])

            def aggregate(fulls, FD, ht, b_sb):
                """gather full-width rows by edge sources (one DMA row per
                edge), node-major reduce via S matmuls, tanh on the full
                2048-wide row (one Act op per block), then transpose into
                feature-major ht.  All half-A work first (gates only on
                AllGather A); partials park in SBUF (with the bias folded
                in) and fold into the half-B sums."""
                with (
                    tc.tile_pool(name=pname("ag"), bufs=1) as apool,
                    tc.tile_pool(name=pname("agps"), bufs=1, space="PSUM") as aps,
                ):
                    a_part = apool.tile([128, NB, FD], BF, tag="apart")
                    for h in range(2):
                        for blk in range(NB):
                            t0 = blk * 2 * CHH + h * CHH
                            g = apool.tile([128, CHH, FD], BF, tag="g", bufs=2)
                            nc.gpsimd.dma_gather(
                                g[:], fulls[h][:],
                                idx2_sb[:, t0 * 8:(t0 + CHH) * 8],
                                CHH * 128, CHH * 128, FD, single_packet=False)
                            ps = aps.tile([128, FD], DT, tag="agg", bufs=1)
                            for c in range(CHH):
                                for n0 in range(0, FD, 512):
                                    nc.tensor.matmul(
                                        ps[:, n0:n0 + 512],
                                        s2_sb[:, t0 + c, :],
                                        g[:, c, n0:n0 + 512],
                                        start=(c == 0), stop=(c == CHH - 1))
                            if h == 0:
                                if b_sb is None:
                                    nc.vector.tensor_copy(
                                        a_part[:, blk, :], ps[:])
                                else:
                                    nc.vector.tensor_tensor(
                                        out=a_part[:, blk, :], in0=ps[:],
                                        in1=b_sb[:, :FD],
                                        op=mybir.AluOpType.add)
                            else:
                                a_nm = apool.tile([128, FD], BF, tag="anm",
                                                  bufs=2)
                                nc.vector.tensor_tensor(
                                    out=a_nm[:], in0=ps[:],
                                    in1=a_part[:, blk, :],
                                    op=mybir.AluOpType.add)
                                a_th = apool.tile([128, FD], BF, tag="ath",
                                                  bufs=2)
                                nc.scalar.activation(a_th[:], a_nm[:], TANH)
                                for f in range(FD // 128):
                                    pt = aps.tile([128, 128], BF, tag="pt",
                                                  bufs=4)
                                    nc.tensor.transpose(
                                        pt[:], a_th[:, f * 128:(f + 1) * 128],
                                        id_sb[:])
                                    nc.vector.tensor_copy(
                                        ht[:, f, blk * 128:(blk + 1) * 128],
                                        pt[:])

            def phase_FIN(h3t, wlt_sb):
                with (
                    tc.tile_pool(name=pname("fin"), bufs=1) as fpool,
                    tc.tile_pool(name=pname("finps"), bufs=1, space="PSUM") as fps,
                ):
                    for r in range(NB):
                        ps = fps.tile([128, F_OUT], DT, tag="xw", bufs=2)
                        for k in range(H3 // 128):
                            for n0 in range(0, F_OUT, 512):
                                n1 = min(n0 + 512, F_OUT)
                                nc.tensor.matmul(
                                    ps[:, n0:n1],
                                    h3t[:, k, r * 128:(r + 1) * 128],
                                    wlt_sb[:, k, n0:n1],
                                    start=(k == 0), stop=(k == H3 // 128 - 1))
                        o = fpool.tile([128, F_OUT], DT, tag="o", bufs=3)
                        nc.vector.tensor_tensor(
                            out=o[:], in0=ps[:], in1=bl_sb[:],
                            op=mybir.AluOpType.add)
                        nc.sync.dma_start(out[r * 128:(r + 1) * 128, :], o[:])

            # weight prefetch: w2t and w3t load at t=0 (the L1 gathers are
            # descriptor-rate-bound and leave HBM bandwidth free); wlt loads
            # during the L3 aggregate.  LIFO pool nesting: w3 > w2 > h1t.
            with tc.tile_pool(name=pname("w3"), bufs=1) as w3pool:
                w3t_sb = w3pool.tile([128, H2 // 128, H3], BF, tag="w3t")
                nc.sync.dma_start(w3t_sb[:], w3t_in[:])
                with tc.tile_pool(name=pname("w2"), bufs=1) as w2pool:
                    w2t_sb = w2pool.tile([128, H1 // 128, H2], BF, tag="w2t")
                    nc.sync.dma_start(w2t_sb[:], w2t_in[:])
                    with tc.tile_pool(name=pname("h1t"), bufs=1) as h1t_pool:
                        h1t = h1t_pool.tile([128, H1 // 128, R], BF, tag="h1t")
                        phase_L1(h1t)
                        transform(h1t, H1, H2, w2t_sb, xw2_locs, xw2_fulls)
                with tc.tile_pool(name=pname("h2t"), bufs=1) as h2t_pool:
                    h2t = h2t_pool.tile([128, H2 // 128, R], BF, tag="h2t")
                    aggregate(xw2_fulls, H2, h2t, b2_sb)
                    transform(h2t, H2, H3, w3t_sb, xw3_locs, xw3_fulls)
                with tc.tile_pool(name=pname("h3t"), bufs=1) as h3t_pool:
                    h3t = h3t_pool.tile([128, H3 // 128, R], BF, tag="h3t")
                    wlt_sb = h3t_pool.tile([128, H3 // 128, F_OUT], BF,
                                           tag="wlt")
                    nc.sync.dma_start(wlt_sb[:], wlt_in[:])
                    aggregate(xw3_fulls, H3, h3t, b3_sb)
                    phase_FIN(h3t, wlt_sb)

    nc.compile()
    return nc


# ----------------------------------------------------------------------------
# Entry point
# ----------------------------------------------------------------------------

def _make_in_maps(inputs, perm, pre):
    import ml_dtypes
    bf = ml_dtypes.bfloat16
    _, _, _, idx1_tabs, s1_tabs, idx2_tabs, s2_tabs = pre

    def tile_w(w):  # [K, F] -> [128, K/128, F]
        k, f = w.shape
        return np.ascontiguousarray(
            w.reshape(k // 128, 128, f).transpose(1, 0, 2)).astype(bf)

    x_perm = np.ascontiguousarray(
        np.asarray(inputs["x"], np.float32)[perm]).astype(bf)
    w1t = tile_w(np.ascontiguousarray(np.asarray(inputs["W1"], np.float32).T))
    w2t = tile_w(np.ascontiguousarray(np.asarray(inputs["W2"], np.float32).T))
    w3t = tile_w(np.ascontiguousarray(np.asarray(inputs["W3"], np.float32).T))
    wlt = tile_w(np.ascontiguousarray(np.asarray(inputs["Wl"], np.float32).T))
    b1pp = np.ascontiguousarray(
        np.asarray(inputs["b1"], np.float32).reshape(-1, 128).T)
    b2 = np.asarray(inputs["b2"], np.float32)
    b3 = np.asarray(inputs["b3"], np.float32)
    zero_b23 = not (b2.any() or b3.any())
    blb = np.ascontiguousarray(
        np.broadcast_to(np.asarray(inputs["bl"], np.float32), (128, F_OUT)))
    ident = np.eye(128, dtype=bf)

    in_maps = []
    for c in range(NUM_CORES):
        m = {
            "x_perm": x_perm,
            "idx1_in": idx1_tabs[c], "s1_in": s1_tabs[c].astype(bf),
            "idx2_in": idx2_tabs[c], "s2_in": s2_tabs[c].astype(bf),
            "ident": ident,
            "w1t": w1t, "w2t": w2t, "w3t": w3t, "wlt": wlt,
            "b1pp": b1pp, "blb": blb,
        }
        if not zero_b23:
            m["b2row"] = np.ascontiguousarray(np.broadcast_to(b2, (128, H2)))
            m["b3row"] = np.ascontiguousarray(np.broadcast_to(b3, (128, H3)))
        in_maps.append(m)
    return in_maps, zero_b23


def _run(inputs, trace=False):
    pre = _preprocess(np.asarray(inputs["edge_index"]))
    perm, CH1, CHH = pre[0], pre[1], pre[2]
    in_maps, zero_b23 = _make_in_maps(inputs, perm, pre)
    nc = _build_program(CH1, CHH, zero_b23=zero_b23)
    res = run_bass_kernel_spmd(nc, in_maps, list(range(NUM_CORES)), trace=trace)
    out_perm = np.concatenate([res.results[c]["out"] for c in range(NUM_CORES)], 0)
    out = np.empty_like(out_perm)
    out[perm] = out_perm
    return out, res


def kernel(**inputs):
    out, _ = _run(inputs, trace=False)
    return out


# revision 41
# speedup vs baseline: 1.0974x; 1.0334x over previous
"""3-layer GCN + linear head on 8 Trainium2 NeuronCores.

Sharding: nodes are partitioned across the 8 cores (graph parallel), after a
host-side balanced permutation that gives every 128-node block exactly the
same number of incoming edges (including self loops).  All message traffic is
bf16 (tolerance is 2e-2); PSUM accumulation is fp32.

Per layer (2, 3) each core:
  - transforms its local rows 0-511 (dense matmul, weights replicated),
    AllGathers them into fullA [4096, FD] while transforming rows 512-1023,
    which AllGather into fullB,
  - gathers edge-source rows (full-width, 1 DMA descriptor per edge) with
    SWDGE dma_gather and reduces them into destination rows with TensorE
    matmuls against host-built per-chunk selection matrices S (which carry
    the GCN edge normalization weights).  All A-half gathers for the 8 dst
    blocks run first (they only gate on AllGather #1, overlapping AllGather
    #2); partial sums park in SBUF and are folded in during the B half.
Layer 1 aggregates x first (256-dim messages) and transforms after.  The
final linear head has no aggregation.
"""
import sys
if "/opt/trn_rl_repo" not in sys.path:
    sys.path.insert(0, "/opt/trn_rl_repo")

import numpy as np

import concourse.bass as bass
import concourse.mybir as mybir
import concourse.tile as tile
from concourse import bacc
from concourse.bass_utils import run_bass_kernel_spmd
from concourse.library_config import mlp

N = 8192
NUM_CORES = 8
R = N // NUM_CORES          # rows per core
HALF = R // 2               # rows per AllGather shard
NB = 8                      # dst blocks per core (128 rows each)
NBINS = NUM_CORES * NB
BIN_SZ = 128
F_IN, H1, H2, H3, F_OUT = 256, 2048, 2048, 1024, 768
DT = mybir.dt.float32
BF = mybir.dt.bfloat16
TANH = mybir.ActivationFunctionType.Tanh


# ----------------------------------------------------------------------------
# Host-side graph preprocessing
# ----------------------------------------------------------------------------

def _pack_idx(chunk_lists, tot_ch):
    """Pack per-chunk [128] source-row arrays into the SWDGE idx layout:
    [128, tot_ch * 8] int16, indices wrapped in 16 partitions and replicated
    8x across partition groups."""
    tab = np.zeros((128, tot_ch * 8), np.int16)
    ar = np.arange(128)
    for t, rows in enumerate(chunk_lists):
        tab[ar % 16, t * 8 + ar // 16] = rows.astype(np.int16)
    for rep in range(1, 8):
        tab[rep * 16:(rep + 1) * 16, :] = tab[:16, :]
    return tab


def _preprocess(edge_index):
    src = np.asarray(edge_index[0], dtype=np.int64)
    dst = np.asarray(edge_index[1], dtype=np.int64)

    deg = np.bincount(dst, minlength=N).astype(np.float64) + 1.0
    dinv = 1.0 / np.sqrt(deg)
    d_in = np.bincount(dst, minlength=N) + 1

    # greedy balanced partition of nodes into bins of 128, equal in-edge sums
    order = np.argsort(-d_in, kind="stable")
    bin_sum = np.zeros(NBINS, dtype=np.int64)
    bin_cnt = np.zeros(NBINS, dtype=np.int64)
    bin_nodes = [[] for _ in range(NBINS)]
    for node in order:
        avail = np.where(bin_cnt < BIN_SZ)[0]
        b = avail[np.argmin(bin_sum[avail])]
        bin_nodes[b].append(node)
        bin_sum[b] += d_in[node]
        bin_cnt[b] += 1

    target = int(np.ceil(d_in.sum() / NBINS))
    for _ in range(200):
        hi = int(np.argmax(bin_sum))
        if bin_sum[hi] <= target:
            break
        lo = int(np.argmin(bin_sum))
        need = bin_sum[hi] - target
        best = None
        for ai, a in enumerate(bin_nodes[hi]):
            for bi, b in enumerate(bin_nodes[lo]):
                diff = d_in[a] - d_in[b]
                if diff > 0:
                    score = abs(diff - need)
                    if best is None or score < best[0]:
                        best = (score, ai, bi)
        if best is None:
            break
        _, ai, bi = best
        a, b = bin_nodes[hi][ai], bin_nodes[lo][bi]
        bin_nodes[hi][ai], bin_nodes[lo][bi] = b, a
        bin_sum[hi] += d_in[b] - d_in[a]
        bin_sum[lo] += d_in[a] - d_in[b]

    CH1 = int(np.ceil(bin_sum.max() / 128))

    # Assign bins to slots so that no (dst block, src half) pair has more
    # than 5*128 edges: fullA/fullB each hold the union of every core's
    # half-A/half-B bins (slot%8 < 4 -> A), the L2/L3 gathers pad each
    # (block, half) to a multiple of 128 rows, and the assignment is free.
    src_bin = np.empty(N, np.int64)
    for i, bn in enumerate(bin_nodes):
        src_bin[np.array(bn, dtype=np.int64)] = i
    es = np.concatenate([src, np.arange(N, dtype=np.int64)])
    ed = np.concatenate([dst, np.arange(N, dtype=np.int64)])
    C = np.zeros((NBINS, NBINS), np.int64)
    np.add.at(C, (src_bin[ed], src_bin[es]), 1)

    # any 32/32 partition of bins into halves works (within-core slot
    # order is free): random restarts + swap hill-climb on max halfsum
    rng = np.random.default_rng(0)
    tot = C.sum(axis=1)

    def climb(memb, iters):
        MA = C[:, memb == 0].sum(axis=1)
        best = int(np.maximum(MA, tot - MA).max())
        for _ in range(iters):
            if best <= 640:
                break
            ia = rng.choice(np.where(memb == 0)[0])
            ib = rng.choice(np.where(memb == 1)[0])
            cand = MA + C[:, ib] - C[:, ia]
            mx = int(np.maximum(cand, tot - cand).max())
            if mx <= best:
                best = mx
                MA = cand
                memb[ia], memb[ib] = 1, 0
        return best, memb

    best_mx, best_memb = None, None
    for _ in range(300):
        memb = np.zeros(NBINS, np.int8)
        memb[rng.permutation(NBINS)[:NBINS // 2]] = 1
        MA = C[:, memb == 0].sum(axis=1)
        mx = int(np.maximum(MA, tot - MA).max())
        if best_mx is None or mx < best_mx:
            best_mx, best_memb = mx, memb.copy()
        if best_mx <= 600:
            break
    if best_mx > 600:
        best_mx, best_memb = climb(best_memb, 5000)

    # node-level refinement: swap equal-in-degree nodes across halves
    # (keeps every bin sum exact, so CH1 is unaffected) until no
    # (block, half) pair exceeds 5*128 edges
    node_bin = src_bin.copy()

    def half_counts(nb):
        m2 = np.zeros((NBINS, 2), np.int64)
        np.add.at(m2, (nb[ed], best_memb[nb[es]].astype(np.int64)), 1)
        return m2

    def m2_score(m2):
        mx = int(m2.max())
        return (mx, int((m2 > 640).sum()), int(m2[m2 > 640].sum()))

    m2 = half_counts(node_bin)
    cur = m2_score(m2)
    d_all = d_in  # includes the self loop
    for _ in range(400):
        if cur[0] <= 640:
            break
        b_star, h_star = np.unravel_index(np.argmax(m2), m2.shape)
        # sources (in half h_star) of edges into b_star
        cand = es[(node_bin[ed] == b_star)
                  & (best_memb[node_bin[es]] == h_star)]
        u = int(rng.choice(cand))
        other = np.where((best_memb[node_bin] != h_star)
                         & (d_all == d_all[u]))[0]
        if len(other) == 0:
            continue
        v = int(rng.choice(other))
        node_bin[u], node_bin[v] = node_bin[v], node_bin[u]
        m2_new = half_counts(node_bin)
        new = m2_score(m2_new)
        if new <= cur:
            cur, m2 = new, m2_new
        else:
            node_bin[u], node_bin[v] = node_bin[v], node_bin[u]

    bin_nodes = [list(np.where(node_bin == i)[0]) for i in range(NBINS)]
    a_bins = list(np.where(best_memb == 0)[0])
    b_bins = list(np.where(best_memb == 1)[0])
    slots = np.empty(NBINS, np.int64)
    for c in range(NUM_CORES):
        slots[c * 8:c * 8 + 4] = a_bins[c * 4:(c + 1) * 4]
        slots[c * 8 + 4:c * 8 + 8] = b_bins[c * 4:(c + 1) * 4]
    bin_nodes = [bin_nodes[s] for s in slots]
    bin_sum = np.array([int(d_in[bn].sum()) for bn in bin_nodes])

    perm = np.concatenate([np.array(bn, dtype=np.int64) for bn in bin_nodes])
    inv = np.empty(N, dtype=np.int64)
    inv[perm] = np.arange(N)

    all_src = np.concatenate([inv[src], np.arange(N, dtype=np.int64)])
    all_dst = np.concatenate([inv[dst], np.arange(N, dtype=np.int64)])
    all_w = np.concatenate([
        (dinv[src] * dinv[dst]).astype(np.float32),
        (dinv[perm] * dinv[perm]).astype(np.float32),
    ])

    bin_of = all_dst // BIN_SZ
    dst_local = all_dst % BIN_SZ
    src_half = (all_src % R) // HALF
    src_hrow = (all_src // R) * HALF + (all_src % HALF)

    # CHH: chunks per (block, half) for the L2/L3 gathers
    CHH = 0
    for b in range(NBINS):
        for h in range(2):
            n = int(np.sum((bin_of == b) & (src_half == h)))
            CHH = max(CHH, (n + 127) // 128)

    idx1_tabs, s1_tabs, idx2_tabs, s2_tabs = [], [], [], []
    for c in range(NUM_CORES):
        chunks1, chunks2 = [], []
        s1 = np.zeros((128, NB * CH1, 128), np.float32)
        s2 = np.zeros((128, NB * 2 * CHH, 128), np.float32)
        for blk in range(NB):
            sel = np.where(bin_of == c * NB + blk)[0]
            # L1 table: all edges of the block, sources are x_perm rows
            e_src = np.zeros(CH1 * 128, np.int64)
            e_dst = np.zeros(CH1 * 128, np.int64)
            e_w = np.zeros(CH1 * 128, np.float32)
            e_src[:len(sel)] = all_src[sel]
            e_dst[:len(sel)] = dst_local[sel]
            e_w[:len(sel)] = all_w[sel]
            for ci in range(CH1):
                sl = slice(ci * 128, (ci + 1) * 128)
                chunks1.append(e_src[sl])
                s1[np.arange(128), blk * CH1 + ci, e_dst[sl]] = e_w[sl]
            # L2/L3 table: edges split by source half, row ids in half tensor
            for h in range(2):
                selh = sel[src_half[sel] == h]
                e_src = np.zeros(CHH * 128, np.int64)
                e_dst = np.zeros(CHH * 128, np.int64)
                e_w = np.zeros(CHH * 128, np.float32)
                e_src[:len(selh)] = src_hrow[selh]
                e_dst[:len(selh)] = dst_local[selh]
                e_w[:len(selh)] = all_w[selh]
                for ci in range(CHH):
                    sl = slice(ci * 128, (ci + 1) * 128)
                    chunks2.append(e_src[sl])
                    t = blk * 2 * CHH + h * CHH + ci
                    s2[np.arange(128), t, e_dst[sl]] = e_w[sl]
        idx1_tabs.append(_pack_idx(chunks1, NB * CH1))
        s1_tabs.append(s1)
        idx2_tabs.append(_pack_idx(chunks2, NB * 2 * CHH))
        s2_tabs.append(s2)

    return perm, CH1, CHH, idx1_tabs, s1_tabs, idx2_tabs, s2_tabs


# ----------------------------------------------------------------------------
# Device program
# ----------------------------------------------------------------------------

def _build_program(CH1, CHH, zero_b23=True):
    T1 = NB * CH1
    T2 = NB * 2 * CHH
    nc = bacc.Bacc("TRN2", target_bir_lowering=False, debug=False,
                   num_devices=NUM_CORES)
    core_ids = list(range(NUM_CORES))

    x_perm = nc.dram_tensor("x_perm", [N, F_IN], BF, kind="ExternalInput")
    idx1_in = nc.dram_tensor("idx1_in", [128, T1 * 8], mybir.dt.int16,
                             kind="ExternalInput")
    s1_in = nc.dram_tensor("s1_in", [128, T1, 128], BF, kind="ExternalInput")
    idx2_in = nc.dram_tensor("idx2_in", [128, T2 * 8], mybir.dt.int16,
                             kind="ExternalInput")
    s2_in = nc.dram_tensor("s2_in", [128, T2, 128], BF, kind="ExternalInput")
    ident_in = nc.dram_tensor("ident", [128, 128], BF, kind="ExternalInput")
    # weights pre-tiled on host to [128, K/128, F] layout
    w1t_in = nc.dram_tensor("w1t", [128, F_IN // 128, H1], BF, kind="ExternalInput")
    w2t_in = nc.dram_tensor("w2t", [128, H1 // 128, H2], BF, kind="ExternalInput")
    w3t_in = nc.dram_tensor("w3t", [128, H2 // 128, H3], BF, kind="ExternalInput")
    wlt_in = nc.dram_tensor("wlt", [128, H3 // 128, F_OUT], BF, kind="ExternalInput")
    b1_in = nc.dram_tensor("b1pp", [128, H1 // 128], DT, kind="ExternalInput")
    if not zero_b23:
        b2_in = nc.dram_tensor("b2row", [128, H2], DT, kind="ExternalInput")
        b3_in = nc.dram_tensor("b3row", [128, H3], DT, kind="ExternalInput")
    bl_in = nc.dram_tensor("blb", [128, F_OUT], DT, kind="ExternalInput")

    out = nc.dram_tensor("out", [R, F_OUT], DT, kind="ExternalOutput")

    xw2_locs = [nc.dram_tensor(f"xw2_loc{h}", [HALF, H2], BF) for h in range(2)]
    xw2_fulls = [nc.dram_tensor(f"xw2_full{h}", [NUM_CORES * HALF, H2], BF,
                                addr_space="Shared") for h in range(2)]
    xw3_locs = [nc.dram_tensor(f"xw3_loc{h}", [HALF, H3], BF) for h in range(2)]
    xw3_fulls = [nc.dram_tensor(f"xw3_full{h}", [NUM_CORES * HALF, H3], BF,
                                addr_space="Shared") for h in range(2)]

    uid = [0]

    def pname(base):
        uid[0] += 1
        return f"{base}{uid[0]}"

    with tile.TileContext(nc) as tc:
        with tc.tile_pool(name="const", bufs=1) as cpool:
            nc.gpsimd.load_library(mlp)
            idx2_sb = cpool.tile([128, T2 * 8], mybir.dt.int16, tag="idx2")
            s2_sb = cpool.tile([128, T2, 128], BF, tag="s2")
            id_sb = cpool.tile([128, 128], BF, tag="ident")
            nc.sync.dma_start(id_sb[:], ident_in[:])
            b1_sb = cpool.tile([128, H1 // 128], DT, tag="b1")
            nc.sync.dma_start(b1_sb[:], b1_in[:])
            if zero_b23:
                b2_sb = b3_sb = None
            else:
                b2_sb = cpool.tile([128, H2], DT, tag="b2")
                nc.sync.dma_start(b2_sb[:], b2_in[:])
                b3_sb = cpool.tile([128, H3], DT, tag="b3")
                nc.sync.dma_start(b3_sb[:], b3_in[:])
            bl_sb = cpool.tile([128, F_OUT], DT, tag="bl")
            nc.sync.dma_start(bl_sb[:], bl_in[:])

            def phase_L1(h1t, late_loads):
                """gather x, aggregate node-major, transpose, transform+tanh.
                late_loads: (sbuf_tile, dram) pairs emitted on the sync queue
                AFTER the L1 gather tables, so the first gathers are not
                stuck behind multi-MB weight prefetches."""
                with (
                    tc.tile_pool(name=pname("l1a"), bufs=1) as l1a_pool,
                    tc.tile_pool(name=pname("l1ps"), bufs=1, space="PSUM") as l1ps,
                ):
                    idx1_sb = l1a_pool.tile([128, T1 * 8], mybir.dt.int16,
                                            tag="idx1")
                    nc.sync.dma_start(idx1_sb[:], idx1_in[:])
                    s1_sb = l1a_pool.tile([128, T1, 128], BF, tag="s1")
                    nc.sync.dma_start(s1_sb[:], s1_in[:])
                    w1t_sb = l1a_pool.tile([128, F_IN // 128, H1], BF, tag="w1t")
                    for t, dr in late_loads:
                        nc.sync.dma_start(t[:], dr[:])
                    nc.sync.dma_start(w1t_sb[:], w1t_in[:])
                    agg1t = l1a_pool.tile([128, F_IN // 128, R], BF, tag="agg1t")
                    for blk in range(NB):
                        g = l1a_pool.tile([128, CH1, F_IN], BF, tag="g1", bufs=2)
                        nc.gpsimd.dma_gather(
                            g[:], x_perm[:],
                            idx1_sb[:, blk * CH1 * 8:(blk + 1) * CH1 * 8],
                            CH1 * 128, CH1 * 128, F_IN, single_packet=False)
                        ps = l1ps.tile([128, F_IN], DT, tag="agg", bufs=2)
                        for c in range(CH1):
                            nc.tensor.matmul(
                                ps[:, :], s1_sb[:, blk * CH1 + c, :], g[:, c, :],
                                start=(c == 0), stop=(c == CH1 - 1))
                        a_nm = l1a_pool.tile([128, F_IN], BF, tag="anm", bufs=2)
                        nc.vector.tensor_copy(a_nm[:], ps[:])
                        for f in range(F_IN // 128):
                            pt = l1ps.tile([128, 128], BF, tag="pt", bufs=2)
                            nc.tensor.transpose(
                                pt[:], a_nm[:, f * 128:(f + 1) * 128], id_sb[:])
                            nc.vector.tensor_copy(
                                agg1t[:, f, blk * 128:(blk + 1) * 128], pt[:])
                    for m in range(H1 // 128):
                        ps = l1ps.tile([128, R], DT, tag="xw", bufs=2)
                        for k in range(F_IN // 128):
                            for n in range(0, R, 512):
                                nc.tensor.matmul(
                                    ps[:, n:n + 512],
                                    w1t_sb[:, k, m * 128:(m + 1) * 128],
                                    agg1t[:, k, n:n + 512],
                                    start=(k == 0), stop=(k == F_IN // 128 - 1))
                        nc.scalar.activation(
                            h1t[:, m, :], ps[:], TANH, bias=b1_sb[:, m:m + 1])

            def transform_block(ht, HK, FD, wt_sb, locs, r, tpool, tps,
                                ps_bufs=2):
                """one 128-row block of (ht rows) @ W^T into locs[r//4]."""
                h, r4 = divmod(r, HALF // 128)
                ps = tps.tile([128, FD], DT, tag="xw", bufs=ps_bufs)
                for k in range(HK):
                    for n0 in range(0, FD, 512):
                        n1 = min(n0 + 512, FD)
                        nc.tensor.matmul(
                            ps[:, n0:n1],
                            ht[:, k, r * 128:(r + 1) * 128],
                            wt_sb[:, k, n0:n1],
                            start=(k == 0), stop=(k == HK - 1))
                o = tpool.tile([128, FD], BF, tag="o", bufs=3)
                nc.vector.tensor_copy(o[:], ps[:])
                nc.sync.dma_start(locs[h][r4 * 128:(r4 + 1) * 128, :], o[:])

            def allgather(locs, fulls, h):
                nc.gpsimd.collective_compute(
                    "AllGather", mybir.AluOpType.bypass,
                    replica_groups=[core_ids],
                    ins=[locs[h][:]], outs=[fulls[h][:]])

            def transform(ht, KD, FD, wt_sb, locs, fulls):
                """locs[h] = rows [h*512, h*512+512) of (ht rows) @ W^T;
                AllGather each row-half as soon as it is written so the
                collective overlaps with the other half's matmuls."""
                with (
                    tc.tile_pool(name=pname("tr"), bufs=1) as tpool,
                    tc.tile_pool(name=pname("trps"), bufs=1, space="PSUM") as tps,
                ):
                    for h in range(2):
                        for r4 in range(HALF // 128):
                            transform_block(ht, KD // 128, FD, wt_sb, locs,
                                            h * (HALF // 128) + r4, tpool, tps)
                        allgather(locs, fulls, h)

            def aggregate(fulls, FD, ht, b_sb, post_blk=None):
                """gather full-width rows by edge sources (one DMA row per
                edge), node-major reduce via S matmuls, tanh on the full
                FD-wide row (one Act op per block), then transpose into
                feature-major ht.  All half-A work first (gates only on
                AllGather A); partials park in SBUF (with the bias folded
                in) and fold into the half-B sums.  The epilogue transposes
                (and post_blk: the next matmul stage for that dst block) are
                emitted one block late so the in-order PE queue never stalls
                waiting on the DVE/Act producers of a_th."""
                with (
                    tc.tile_pool(name=pname("ag"), bufs=1) as apool,
                    tc.tile_pool(name=pname("agps"), bufs=1, space="PSUM") as aps,
                ):
                    a_part = apool.tile([128, NB, FD], BF, tag="apart")

                    def epilogue(blk, a_th):
                        for f in range(FD // 128):
                            pt = aps.tile([128, 128], BF, tag="pt", bufs=2)
                            nc.tensor.transpose(
                                pt[:], a_th[:, f * 128:(f + 1) * 128],
                                id_sb[:])
                            nc.vector.tensor_copy(
                                ht[:, f, blk * 128:(blk + 1) * 128], pt[:])
                        if post_blk is not None:
                            post_blk(blk)

                    pending = None
                    for h in range(2):
                        for blk in range(NB):
                            t0 = blk * 2 * CHH + h * CHH
                            g = apool.tile([128, CHH, FD], BF, tag="g", bufs=2)
                            nc.gpsimd.dma_gather(
                                g[:], fulls[h][:],
                                idx2_sb[:, t0 * 8:(t0 + CHH) * 8],
                                CHH * 128, CHH * 128, FD, single_packet=False)
                            ps = aps.tile([128, FD], DT, tag="agg", bufs=1)
                            for c in range(CHH):
                                for n0 in range(0, FD, 512):
                                    nc.tensor.matmul(
                                        ps[:, n0:n0 + 512],
                                        s2_sb[:, t0 + c, :],
                                        g[:, c, n0:n0 + 512],
                                        start=(c == 0), stop=(c == CHH - 1))
                            if h == 0:
                                if b_sb is None:
                                    nc.vector.tensor_copy(
                                        a_part[:, blk, :], ps[:])
                                else:
                                    nc.vector.tensor_tensor(
                                        out=a_part[:, blk, :], in0=ps[:],
                                        in1=b_sb[:, :FD],
                                        op=mybir.AluOpType.add)
                            else:
                                a_nm = apool.tile([128, FD], BF, tag="anm",
                                                  bufs=2)
                                nc.vector.tensor_tensor(
                                    out=a_nm[:], in0=ps[:],
                                    in1=a_part[:, blk, :],
                                    op=mybir.AluOpType.add)
                                a_th = apool.tile([128, FD], BF, tag="ath",
                                                  bufs=3)
                                nc.scalar.activation(a_th[:], a_nm[:], TANH)
                                if pending is not None:
                                    epilogue(*pending)
                                pending = (blk, a_th)
                    epilogue(*pending)

            def fin_block(h3t, wlt_sb, r, fpool, fps):
                ps = fps.tile([128, F_OUT], DT, tag="xw", bufs=1)
                for k in range(H3 // 128):
                    for n0 in range(0, F_OUT, 512):
                        n1 = min(n0 + 512, F_OUT)
                        nc.tensor.matmul(
                            ps[:, n0:n1],
                            h3t[:, k, r * 128:(r + 1) * 128],
                            wlt_sb[:, k, n0:n1],
                            start=(k == 0), stop=(k == H3 // 128 - 1))
                o = fpool.tile([128, F_OUT], DT, tag="o", bufs=3)
                nc.vector.tensor_tensor(
                    out=o[:], in0=ps[:], in1=bl_sb[:],
                    op=mybir.AluOpType.add)
                nc.sync.dma_start(out[r * 128:(r + 1) * 128, :], o[:])

            # weight prefetch: w2t/w3t and the s2/idx2 tables load right
            # after the L1 gather tables (the L1 gathers are descriptor-
            # rate-bound and leave HBM bandwidth free); wlt loads during
            # the L2 aggregate.  LIFO pool nesting: w3 > w2 > h1t.
            with tc.tile_pool(name=pname("w3"), bufs=1) as w3pool:
                w3t_sb = w3pool.tile([128, H2 // 128, H3], BF, tag="w3t")
                with tc.tile_pool(name=pname("w2"), bufs=1) as w2pool:
                    w2t_sb = w2pool.tile([128, H1 // 128, H2], BF, tag="w2t")
                    with tc.tile_pool(name=pname("h1t"), bufs=1) as h1t_pool:
                        h1t = h1t_pool.tile([128, H1 // 128, R], BF, tag="h1t")
                        phase_L1(h1t, [
                            (w2t_sb, w2t_in), (w3t_sb, w3t_in),
                            (idx2_sb, idx2_in), (s2_sb, s2_in)])
                        transform(h1t, H1, H2, w2t_sb, xw2_locs, xw2_fulls)
                with tc.tile_pool(name=pname("h2t"), bufs=1) as h2t_pool:
                    h2t = h2t_pool.tile([128, H2 // 128, R], BF, tag="h2t")
                    wlt_sb = h2t_pool.tile([128, H3 // 128, F_OUT], BF,
                                           tag="wlt")
                    nc.sync.dma_start(wlt_sb[:], wlt_in[:])
                    with (
                        tc.tile_pool(name=pname("l3t"), bufs=1) as l3tp,
                        tc.tile_pool(name=pname("l3tps"), bufs=1,
                                     space="PSUM") as l3tps,
                    ):
                        # L3 transform blocks are emitted inside the L2
                        # aggregate's B half (one block behind the S-matmul
                        # groups), so the PE stays continuously busy; the
                        # AllGather triggers follow the whole loop so the
                        # gpsimd queue's gathers are never stuck behind them.
                        aggregate(xw2_fulls, H2, h2t, b2_sb,
                                  post_blk=lambda r: transform_block(
                                      h2t, H2 // 128, H3, w3t_sb, xw3_locs,
                                      r, l3tp, l3tps, ps_bufs=1))
                    allgather(xw3_locs, xw3_fulls, 0)
                    allgather(xw3_locs, xw3_fulls, 1)
                    with tc.tile_pool(name=pname("h3t"), bufs=1) as h3t_pool:
                        h3t = h3t_pool.tile([128, H3 // 128, R], BF,
                                            tag="h3t")
                        with (
                            tc.tile_pool(name=pname("fin"), bufs=1) as fpool,
                            tc.tile_pool(name=pname("finps"), bufs=1,
                                         space="PSUM") as fps,
                        ):
                            aggregate(xw3_fulls, H3, h3t, b3_sb,
                                      post_blk=lambda r: fin_block(
                                          h3t, wlt_sb, r, fpool, fps))

    nc.compile()
    return nc


# ----------------------------------------------------------------------------
# Entry point
# ----------------------------------------------------------------------------

def _make_in_maps(inputs, perm, pre):
    import ml_dtypes
    bf = ml_dtypes.bfloat16
    _, _, _, idx1_tabs, s1_tabs, idx2_tabs, s2_tabs = pre

    def tile_w(w):  # [K, F] -> [128, K/128, F]
        k, f = w.shape
        return np.ascontiguousarray(
            w.reshape(k // 128, 128, f).transpose(1, 0, 2)).astype(bf)

    x_perm = np.ascontiguousarray(
        np.asarray(inputs["x"], np.float32)[perm]).astype(bf)
    w1t = tile_w(np.ascontiguousarray(np.asarray(inputs["W1"], np.float32).T))
    w2t = tile_w(np.ascontiguousarray(np.asarray(inputs["W2"], np.float32).T))
    w3t = tile_w(np.ascontiguousarray(np.asarray(inputs["W3"], np.float32).T))
    wlt = tile_w(np.ascontiguousarray(np.asarray(inputs["Wl"], np.float32).T))
    b1pp = np.ascontiguousarray(
        np.asarray(inputs["b1"], np.float32).reshape(-1, 128).T)
    b2 = np.asarray(inputs["b2"], np.float32)
    b3 = np.asarray(inputs["b3"], np.float32)
    zero_b23 = not (b2.any() or b3.any())
    blb = np.ascontiguousarray(
        np.broadcast_to(np.asarray(inputs["bl"], np.float32), (128, F_OUT)))
    ident = np.eye(128, dtype=bf)

    in_maps = []
    for c in range(NUM_CORES):
        m = {
            "x_perm": x_perm,
            "idx1_in": idx1_tabs[c], "s1_in": s1_tabs[c].astype(bf),
            "idx2_in": idx2_tabs[c], "s2_in": s2_tabs[c].astype(bf),
            "ident": ident,
            "w1t": w1t, "w2t": w2t, "w3t": w3t, "wlt": wlt,
            "b1pp": b1pp, "blb": blb,
        }
        if not zero_b23:
            m["b2row"] = np.ascontiguousarray(np.broadcast_to(b2, (128, H2)))
            m["b3row"] = np.ascontiguousarray(np.broadcast_to(b3, (128, H3)))
        in_maps.append(m)
    return in_maps, zero_b23


def _run(inputs, trace=False):
    pre = _preprocess(np.asarray(inputs["edge_index"]))
    perm, CH1, CHH = pre[0], pre[1], pre[2]
    in_maps, zero_b23 = _make_in_maps(inputs, perm, pre)
    nc = _build_program(CH1, CHH, zero_b23=zero_b23)
    res = run_bass_kernel_spmd(nc, in_maps, list(range(NUM_CORES)), trace=trace)
    out_perm = np.concatenate([res.results[c]["out"] for c in range(NUM_CORES)], 0)
    out = np.empty_like(out_perm)
    out[perm] = out_perm
    return out, res


def kernel(**inputs):
    out, _ = _run(inputs, trace=False)
    return out


# revision 48
# speedup vs baseline: 1.1205x; 1.0210x over previous
"""3-layer GCN + linear head on 8 Trainium2 NeuronCores.

Sharding: nodes are partitioned across the 8 cores (graph parallel), after a
host-side balanced permutation that gives every 128-node block exactly the
same number of incoming edges (including self loops).  All message traffic is
bf16 (tolerance is 2e-2); PSUM accumulation is fp32.

Per layer (2, 3) each core:
  - transforms its local rows 0-511 (dense matmul, weights replicated),
    AllGathers them into fullA [4096, FD] while transforming rows 512-1023,
    which AllGather into fullB,
  - gathers edge-source rows (full-width, 1 DMA descriptor per edge) with
    SWDGE dma_gather and reduces them into destination rows with TensorE
    matmuls against host-built per-chunk selection matrices S (which carry
    the GCN edge normalization weights).  All A-half gathers for the 8 dst
    blocks run first (they only gate on AllGather #1, overlapping AllGather
    #2); partial sums park in SBUF and are folded in during the B half.
Layer 1 aggregates x first (256-dim messages) and transforms after.  The
final linear head has no aggregation.
"""
import sys
if "/opt/trn_rl_repo" not in sys.path:
    sys.path.insert(0, "/opt/trn_rl_repo")

import numpy as np

import concourse.bass as bass
import concourse.mybir as mybir
import concourse.tile as tile
from concourse import bacc
from concourse.bass_utils import run_bass_kernel_spmd
from concourse.library_config import mlp

N = 8192
NUM_CORES = 8
R = N // NUM_CORES          # rows per core
HALF = R // 2               # rows per AllGather shard
NB = 8                      # dst blocks per core (128 rows each)
NBINS = NUM_CORES * NB
BIN_SZ = 128
F_IN, H1, H2, H3, F_OUT = 256, 2048, 2048, 1024, 768
DT = mybir.dt.float32
BF = mybir.dt.bfloat16
TANH = mybir.ActivationFunctionType.Tanh


# ----------------------------------------------------------------------------
# Host-side graph preprocessing
# ----------------------------------------------------------------------------

def _pack_idx(chunk_lists, tot_ch):
    """Pack per-chunk [128] source-row arrays into the SWDGE idx layout:
    [128, tot_ch * 8] int16, indices wrapped in 16 partitions and replicated
    8x across partition groups."""
    tab = np.zeros((128, tot_ch * 8), np.int16)
    ar = np.arange(128)
    for t, rows in enumerate(chunk_lists):
        tab[ar % 16, t * 8 + ar // 16] = rows.astype(np.int16)
    for rep in range(1, 8):
        tab[rep * 16:(rep + 1) * 16, :] = tab[:16, :]
    return tab


def _preprocess(edge_index):
    src = np.asarray(edge_index[0], dtype=np.int64)
    dst = np.asarray(edge_index[1], dtype=np.int64)

    deg = np.bincount(dst, minlength=N).astype(np.float64) + 1.0
    dinv = 1.0 / np.sqrt(deg)
    d_in = np.bincount(dst, minlength=N) + 1

    # greedy balanced partition of nodes into bins of 128, equal in-edge sums
    order = np.argsort(-d_in, kind="stable")
    bin_sum = np.zeros(NBINS, dtype=np.int64)
    bin_cnt = np.zeros(NBINS, dtype=np.int64)
    bin_nodes = [[] for _ in range(NBINS)]
    for node in order:
        avail = np.where(bin_cnt < BIN_SZ)[0]
        b = avail[np.argmin(bin_sum[avail])]
        bin_nodes[b].append(node)
        bin_sum[b] += d_in[node]
        bin_cnt[b] += 1

    target = int(np.ceil(d_in.sum() / NBINS))
    for _ in range(200):
        hi = int(np.argmax(bin_sum))
        if bin_sum[hi] <= target:
            break
        lo = int(np.argmin(bin_sum))
        need = bin_sum[hi] - target
        best = None
        for ai, a in enumerate(bin_nodes[hi]):
            for bi, b in enumerate(bin_nodes[lo]):
                diff = d_in[a] - d_in[b]
                if diff > 0:
                    score = abs(diff - need)
                    if best is None or score < best[0]:
                        best = (score, ai, bi)
        if best is None:
            break
        _, ai, bi = best
        a, b = bin_nodes[hi][ai], bin_nodes[lo][bi]
        bin_nodes[hi][ai], bin_nodes[lo][bi] = b, a
        bin_sum[hi] += d_in[b] - d_in[a]
        bin_sum[lo] += d_in[a] - d_in[b]

    CH1 = int(np.ceil(bin_sum.max() / 128))

    # Assign bins to slots so that no (dst block, src half) pair has more
    # than 5*128 edges: fullA/fullB each hold the union of every core's
    # half-A/half-B bins (slot%8 < 4 -> A), the L2/L3 gathers pad each
    # (block, half) to a multiple of 128 rows, and the assignment is free.
    src_bin = np.empty(N, np.int64)
    for i, bn in enumerate(bin_nodes):
        src_bin[np.array(bn, dtype=np.int64)] = i
    es = np.concatenate([src, np.arange(N, dtype=np.int64)])
    ed = np.concatenate([dst, np.arange(N, dtype=np.int64)])
    C = np.zeros((NBINS, NBINS), np.int64)
    np.add.at(C, (src_bin[ed], src_bin[es]), 1)

    # any 32/32 partition of bins into halves works (within-core slot
    # order is free): random restarts + swap hill-climb on max halfsum
    rng = np.random.default_rng(0)
    tot = C.sum(axis=1)

    def climb(memb, iters):
        MA = C[:, memb == 0].sum(axis=1)
        best = int(np.maximum(MA, tot - MA).max())
        for _ in range(iters):
            if best <= 640:
                break
            ia = rng.choice(np.where(memb == 0)[0])
            ib = rng.choice(np.where(memb == 1)[0])
            cand = MA + C[:, ib] - C[:, ia]
            mx = int(np.maximum(cand, tot - cand).max())
            if mx <= best:
                best = mx
                MA = cand
                memb[ia], memb[ib] = 1, 0
        return best, memb

    best_mx, best_memb = None, None
    for _ in range(300):
        memb = np.zeros(NBINS, np.int8)
        memb[rng.permutation(NBINS)[:NBINS // 2]] = 1
        MA = C[:, memb == 0].sum(axis=1)
        mx = int(np.maximum(MA, tot - MA).max())
        if best_mx is None or mx < best_mx:
            best_mx, best_memb = mx, memb.copy()
        if best_mx <= 600:
            break
    if best_mx > 600:
        best_mx, best_memb = climb(best_memb, 5000)

    # node-level refinement: swap equal-in-degree nodes across halves
    # (keeps every bin sum exact, so CH1 is unaffected) until no
    # (block, half) pair exceeds 5*128 edges
    node_bin = src_bin.copy()

    def half_counts(nb):
        m2 = np.zeros((NBINS, 2), np.int64)
        np.add.at(m2, (nb[ed], best_memb[nb[es]].astype(np.int64)), 1)
        return m2

    def m2_score(m2):
        mx = int(m2.max())
        return (mx, int((m2 > 640).sum()), int(m2[m2 > 640].sum()))

    m2 = half_counts(node_bin)
    cur = m2_score(m2)
    d_all = d_in  # includes the self loop
    for _ in range(400):
        if cur[0] <= 640:
            break
        b_star, h_star = np.unravel_index(np.argmax(m2), m2.shape)
        # sources (in half h_star) of edges into b_star
        cand = es[(node_bin[ed] == b_star)
                  & (best_memb[node_bin[es]] == h_star)]
        u = int(rng.choice(cand))
        other = np.where((best_memb[node_bin] != h_star)
                         & (d_all == d_all[u]))[0]
        if len(other) == 0:
            continue
        v = int(rng.choice(other))
        node_bin[u], node_bin[v] = node_bin[v], node_bin[u]
        m2_new = half_counts(node_bin)
        new = m2_score(m2_new)
        if new <= cur:
            cur, m2 = new, m2_new
        else:
            node_bin[u], node_bin[v] = node_bin[v], node_bin[u]

    bin_nodes = [list(np.where(node_bin == i)[0]) for i in range(NBINS)]
    a_bins = list(np.where(best_memb == 0)[0])
    b_bins = list(np.where(best_memb == 1)[0])
    slots = np.empty(NBINS, np.int64)
    for c in range(NUM_CORES):
        slots[c * 8:c * 8 + 4] = a_bins[c * 4:(c + 1) * 4]
        slots[c * 8 + 4:c * 8 + 8] = b_bins[c * 4:(c + 1) * 4]
    bin_nodes = [bin_nodes[s] for s in slots]
    bin_sum = np.array([int(d_in[bn].sum()) for bn in bin_nodes])

    perm = np.concatenate([np.array(bn, dtype=np.int64) for bn in bin_nodes])
    inv = np.empty(N, dtype=np.int64)
    inv[perm] = np.arange(N)

    all_src = np.concatenate([inv[src], np.arange(N, dtype=np.int64)])
    all_dst = np.concatenate([inv[dst], np.arange(N, dtype=np.int64)])
    all_w = np.concatenate([
        (dinv[src] * dinv[dst]).astype(np.float32),
        (dinv[perm] * dinv[perm]).astype(np.float32),
    ])

    bin_of = all_dst // BIN_SZ
    dst_local = all_dst % BIN_SZ
    src_half = (all_src % R) // HALF
    src_hrow = (all_src // R) * HALF + (all_src % HALF)

    # CHH: chunks per (block, half) for the L2/L3 gathers
    CHH = 0
    for b in range(NBINS):
        for h in range(2):
            n = int(np.sum((bin_of == b) & (src_half == h)))
            CHH = max(CHH, (n + 127) // 128)

    # L1: dense per-core normalized adjacency Ab[src, dst_local] (bf16),
    # streamed through the PE as 64 [128, 1024] tiles -- no gathers at all
    ab_mats = []
    core_of_dst = all_dst // R
    dst_in_core = all_dst % R
    for c in range(NUM_CORES):
        sel = np.where(core_of_dst == c)[0]
        ab = np.zeros((N, R), np.float32)
        np.add.at(ab, (all_src[sel], dst_in_core[sel]), all_w[sel])
        ab_mats.append(ab)

    idx2_tabs, s2_tabs = [], []
    for c in range(NUM_CORES):
        chunks2 = []
        s2 = np.zeros((128, NB * 2 * CHH, 128), np.float32)
        for blk in range(NB):
            sel = np.where(bin_of == c * NB + blk)[0]
            # L2/L3 table: edges split by source half, row ids in half tensor
            for h in range(2):
                selh = sel[src_half[sel] == h]
                e_src = np.zeros(CHH * 128, np.int64)
                e_dst = np.zeros(CHH * 128, np.int64)
                e_w = np.zeros(CHH * 128, np.float32)
                e_src[:len(selh)] = src_hrow[selh]
                e_dst[:len(selh)] = dst_local[selh]
                e_w[:len(selh)] = all_w[selh]
                for ci in range(CHH):
                    sl = slice(ci * 128, (ci + 1) * 128)
                    chunks2.append(e_src[sl])
                    t = blk * 2 * CHH + h * CHH + ci
                    s2[np.arange(128), t, e_dst[sl]] = e_w[sl]
        idx2_tabs.append(_pack_idx(chunks2, NB * 2 * CHH))
        s2_tabs.append(s2)

    return perm, CH1, CHH, ab_mats, idx2_tabs, s2_tabs


# ----------------------------------------------------------------------------
# Device program
# ----------------------------------------------------------------------------

def _build_program(CH1, CHH, zero_b23=True):
    T1 = NB * CH1
    T2 = NB * 2 * CHH
    nc = bacc.Bacc("TRN2", target_bir_lowering=False, debug=False,
                   num_devices=NUM_CORES)
    core_ids = list(range(NUM_CORES))

    x_in = nc.dram_tensor("x_tr", [128, N // 128, F_IN], BF,
                          kind="ExternalInput")
    ab_in = nc.dram_tensor("ab", [N, R], BF, kind="ExternalInput")
    idx2_in = nc.dram_tensor("idx2_in", [128, T2 * 8], mybir.dt.int16,
                             kind="ExternalInput")
    s2_in = nc.dram_tensor("s2_in", [128, T2, 128], BF, kind="ExternalInput")
    ident_in = nc.dram_tensor("ident", [128, 128], BF, kind="ExternalInput")
    # weights pre-tiled on host to [128, K/128, F] layout
    w1t_in = nc.dram_tensor("w1t", [128, F_IN // 128, H1], BF, kind="ExternalInput")
    w2t_in = nc.dram_tensor("w2t", [128, H1 // 128, H2], BF, kind="ExternalInput")
    w3t_in = nc.dram_tensor("w3t", [128, H2 // 128, H3], BF, kind="ExternalInput")
    wlt_in = nc.dram_tensor("wlt", [128, H3 // 128, F_OUT], BF, kind="ExternalInput")
    b1_in = nc.dram_tensor("b1pp", [128, H1 // 128], DT, kind="ExternalInput")
    if not zero_b23:
        b2_in = nc.dram_tensor("b2row", [128, H2], DT, kind="ExternalInput")
        b3_in = nc.dram_tensor("b3row", [128, H3], DT, kind="ExternalInput")
    bl_in = nc.dram_tensor("blb", [128, F_OUT], DT, kind="ExternalInput")

    out = nc.dram_tensor("out", [R, F_OUT], DT, kind="ExternalOutput")

    xw2_locs = [nc.dram_tensor(f"xw2_loc{h}", [HALF, H2], BF) for h in range(2)]
    xw2_fulls = [nc.dram_tensor(f"xw2_full{h}", [NUM_CORES * HALF, H2], BF,
                                addr_space="Shared") for h in range(2)]
    xw3_locs = [nc.dram_tensor(f"xw3_loc{h}", [HALF, H3], BF) for h in range(2)]
    xw3_fulls = [nc.dram_tensor(f"xw3_full{h}", [NUM_CORES * HALF, H3], BF,
                                addr_space="Shared") for h in range(2)]

    uid = [0]

    def pname(base):
        uid[0] += 1
        return f"{base}{uid[0]}"

    with tile.TileContext(nc) as tc:
        with tc.tile_pool(name="const", bufs=1) as cpool:
            nc.gpsimd.load_library(mlp)
            idx2_sb = cpool.tile([128, T2 * 8], mybir.dt.int16, tag="idx2")
            s2_sb = cpool.tile([128, T2, 128], BF, tag="s2")
            id_sb = cpool.tile([128, 128], BF, tag="ident")
            nc.sync.dma_start(id_sb[:], ident_in[:])
            b1_sb = cpool.tile([128, H1 // 128], DT, tag="b1")
            nc.sync.dma_start(b1_sb[:], b1_in[:])
            if zero_b23:
                b2_sb = b3_sb = None
            else:
                b2_sb = cpool.tile([128, H2], DT, tag="b2")
                nc.sync.dma_start(b2_sb[:], b2_in[:])
                b3_sb = cpool.tile([128, H3], DT, tag="b3")
                nc.sync.dma_start(b3_sb[:], b3_in[:])
            bl_sb = cpool.tile([128, F_OUT], DT, tag="bl")
            nc.sync.dma_start(bl_sb[:], bl_in[:])

            def phase_L1(h1t, late_loads):
                """L1 aggregation as a dense streamed matmul: x tiles are the
                stationary operand and the dense per-core adjacency Ab streams
                through as the moving operand, accumulating the FEATURE-major
                aggregate for all 1024 local dst rows directly in PSUM (no
                gathers, no transposes).  late_loads (weights, L2/L3 tables)
                go on the sync queue after the Ab stream so the stream is
                never starved."""
                NT = N // 128
                with (
                    tc.tile_pool(name=pname("l1a"), bufs=1) as l1a_pool,
                    tc.tile_pool(name=pname("l1ps"), bufs=1, space="PSUM") as l1ps,
                ):
                    w1t_sb = l1a_pool.tile([128, F_IN // 128, H1], BF, tag="w1t")
                    agg1t = l1a_pool.tile([128, F_IN // 128, R], BF, tag="agg1t")
                    out_ft = l1ps.tile([128, F_IN // 128, R], DT, tag="oft")
                    for st in range(NT):
                        xt = l1a_pool.tile([128, F_IN], BF, tag="xt", bufs=4)
                        nc.sync.dma_start(xt[:], x_in[:, st, :])
                        ab_sb = l1a_pool.tile([128, R], BF, tag="ab", bufs=4)
                        nc.sync.dma_start(
                            ab_sb[:], ab_in[st * 128:(st + 1) * 128, :])
                        for fh in range(F_IN // 128):
                            for n0 in range(0, R, 512):
                                nc.tensor.matmul(
                                    out_ft[:, fh, n0:n0 + 512],
                                    xt[:, fh * 128:(fh + 1) * 128],
                                    ab_sb[:, n0:n0 + 512],
                                    start=(st == 0), stop=(st == NT - 1))
                    nc.sync.dma_start(w1t_sb[:], w1t_in[:])
                    for t, dr in late_loads:
                        nc.sync.dma_start(t[:], dr[:])
                    nc.vector.tensor_copy(agg1t[:], out_ft[:])
                    for m in range(H1 // 128):
                        ps = l1ps.tile([128, R], DT, tag="xw", bufs=2)
                        for k in range(F_IN // 128):
                            for n in range(0, R, 512):
                                nc.tensor.matmul(
                                    ps[:, n:n + 512],
                                    w1t_sb[:, k, m * 128:(m + 1) * 128],
                                    agg1t[:, k, n:n + 512],
                                    start=(k == 0), stop=(k == F_IN // 128 - 1))
                        nc.scalar.activation(
                            h1t[:, m, :], ps[:], TANH, bias=b1_sb[:, m:m + 1])

            def transform_block(ht, HK, FD, wt_sb, locs, r, tpool, tps,
                                ps_bufs=2):
                """one 128-row block of (ht rows) @ W^T into locs[r//4]."""
                h, r4 = divmod(r, HALF // 128)
                ps = tps.tile([128, FD], DT, tag="xw", bufs=ps_bufs)
                for k in range(HK):
                    for n0 in range(0, FD, 512):
                        n1 = min(n0 + 512, FD)
                        nc.tensor.matmul(
                            ps[:, n0:n1],
                            ht[:, k, r * 128:(r + 1) * 128],
                            wt_sb[:, k, n0:n1],
                            start=(k == 0), stop=(k == HK - 1))
                o = tpool.tile([128, FD], BF, tag="o", bufs=3)
                nc.vector.tensor_copy(o[:], ps[:])
                nc.sync.dma_start(locs[h][r4 * 128:(r4 + 1) * 128, :], o[:])

            def allgather(locs, fulls, h):
                nc.gpsimd.collective_compute(
                    "AllGather", mybir.AluOpType.bypass,
                    replica_groups=[core_ids],
                    ins=[locs[h][:]], outs=[fulls[h][:]])

            def transform(ht, KD, FD, wt_sb, locs, fulls):
                """locs[h] = rows [h*512, h*512+512) of (ht rows) @ W^T;
                AllGather each row-half as soon as it is written so the
                collective overlaps with the other half's matmuls."""
                with (
                    tc.tile_pool(name=pname("tr"), bufs=1) as tpool,
                    tc.tile_pool(name=pname("trps"), bufs=1, space="PSUM") as tps,
                ):
                    for h in range(2):
                        for r4 in range(HALF // 128):
                            transform_block(ht, KD // 128, FD, wt_sb, locs,
                                            h * (HALF // 128) + r4, tpool, tps)
                        allgather(locs, fulls, h)

            def aggregate(fulls, FD, ht, b_sb, post_blk=None):
                """gather full-width rows by edge sources (one DMA row per
                edge), node-major reduce via S matmuls, tanh on the full
                FD-wide row (one Act op per block), then transpose into
                feature-major ht.  All half-A work first (gates only on
                AllGather A); partials park in SBUF (with the bias folded
                in) and fold into the half-B sums.  The epilogue transposes
                (and post_blk: the next matmul stage for that dst block) are
                emitted one block late so the in-order PE queue never stalls
                waiting on the DVE/Act producers of a_th."""
                with (
                    tc.tile_pool(name=pname("ag"), bufs=1) as apool,
                    tc.tile_pool(name=pname("agps"), bufs=1, space="PSUM") as aps,
                ):
                    a_part = apool.tile([128, NB, FD], BF, tag="apart")

                    def epilogue(blk, a_th):
                        for f in range(FD // 128):
                            pt = aps.tile([128, 128], BF, tag="pt", bufs=2)
                            nc.tensor.transpose(
                                pt[:], a_th[:, f * 128:(f + 1) * 128],
                                id_sb[:])
                            nc.vector.tensor_copy(
                                ht[:, f, blk * 128:(blk + 1) * 128], pt[:])
                        if post_blk is not None:
                            post_blk(blk)

                    pending = None
                    for h in range(2):
                        for blk in range(NB):
                            t0 = blk * 2 * CHH + h * CHH
                            g = apool.tile([128, CHH, FD], BF, tag="g", bufs=2)
                            nc.gpsimd.dma_gather(
                                g[:], fulls[h][:],
                                idx2_sb[:, t0 * 8:(t0 + CHH) * 8],
                                CHH * 128, CHH * 128, FD, single_packet=False)
                            ps = aps.tile([128, FD], DT, tag="agg", bufs=1)
                            for c in range(CHH):
                                for n0 in range(0, FD, 512):
                                    nc.tensor.matmul(
                                        ps[:, n0:n0 + 512],
                                        s2_sb[:, t0 + c, :],
                                        g[:, c, n0:n0 + 512],
                                        start=(c == 0), stop=(c == CHH - 1))
                            if h == 0:
                                if b_sb is None:
                                    nc.vector.tensor_copy(
                                        a_part[:, blk, :], ps[:])
                                else:
                                    nc.vector.tensor_tensor(
                                        out=a_part[:, blk, :], in0=ps[:],
                                        in1=b_sb[:, :FD],
                                        op=mybir.AluOpType.add)
                            else:
                                a_nm = apool.tile([128, FD], BF, tag="anm",
                                                  bufs=2)
                                nc.vector.tensor_tensor(
                                    out=a_nm[:], in0=ps[:],
                                    in1=a_part[:, blk, :],
                                    op=mybir.AluOpType.add)
                                a_th = apool.tile([128, FD], BF, tag="ath",
                                                  bufs=3)
                                nc.scalar.activation(a_th[:], a_nm[:], TANH)
                                if pending is not None:
                                    epilogue(*pending)
                                pending = (blk, a_th)
                    epilogue(*pending)

            def fin_block(h3t, wlt_sb, r, fpool, fps):
                ps = fps.tile([128, F_OUT], DT, tag="xw", bufs=1)
                for k in range(H3 // 128):
                    for n0 in range(0, F_OUT, 512):
                        n1 = min(n0 + 512, F_OUT)
                        nc.tensor.matmul(
                            ps[:, n0:n1],
                            h3t[:, k, r * 128:(r + 1) * 128],
                            wlt_sb[:, k, n0:n1],
                            start=(k == 0), stop=(k == H3 // 128 - 1))
                o = fpool.tile([128, F_OUT], DT, tag="o", bufs=3)
                nc.vector.tensor_tensor(
                    out=o[:], in0=ps[:], in1=bl_sb[:],
                    op=mybir.AluOpType.add)
                nc.sync.dma_start(out[r * 128:(r + 1) * 128, :], o[:])

            # weight prefetch: w2t/w3t and the s2/idx2 tables load right
            # after the L1 gather tables (the L1 gathers are descriptor-
            # rate-bound and leave HBM bandwidth free); wlt loads during
            # the L2 aggregate.  LIFO pool nesting: w3 > w2 > h1t.
            with tc.tile_pool(name=pname("w3"), bufs=1) as w3pool:
                w3t_sb = w3pool.tile([128, H2 // 128, H3], BF, tag="w3t")
                with tc.tile_pool(name=pname("w2"), bufs=1) as w2pool:
                    w2t_sb = w2pool.tile([128, H1 // 128, H2], BF, tag="w2t")
                    with tc.tile_pool(name=pname("h1t"), bufs=1) as h1t_pool:
                        h1t = h1t_pool.tile([128, H1 // 128, R], BF, tag="h1t")
                        phase_L1(h1t, [
                            (w2t_sb, w2t_in), (w3t_sb, w3t_in),
                            (idx2_sb, idx2_in), (s2_sb, s2_in)])
                        transform(h1t, H1, H2, w2t_sb, xw2_locs, xw2_fulls)
                with tc.tile_pool(name=pname("h2t"), bufs=1) as h2t_pool:
                    h2t = h2t_pool.tile([128, H2 // 128, R], BF, tag="h2t")
                    wlt_sb = h2t_pool.tile([128, H3 // 128, F_OUT], BF,
                                           tag="wlt")
                    nc.sync.dma_start(wlt_sb[:], wlt_in[:])
                    with (
                        tc.tile_pool(name=pname("l3t"), bufs=1) as l3tp,
                        tc.tile_pool(name=pname("l3tps"), bufs=1,
                                     space="PSUM") as l3tps,
                    ):
                        # L3 transform blocks are emitted inside the L2
                        # aggregate's B half (one block behind the S-matmul
                        # groups), so the PE stays continuously busy; the
                        # AllGather triggers follow the whole loop so the
                        # gpsimd queue's gathers are never stuck behind them.
                        aggregate(xw2_fulls, H2, h2t, b2_sb,
                                  post_blk=lambda r: transform_block(
                                      h2t, H2 // 128, H3, w3t_sb, xw3_locs,
                                      r, l3tp, l3tps, ps_bufs=1))
                    allgather(xw3_locs, xw3_fulls, 0)
                    allgather(xw3_locs, xw3_fulls, 1)
                    with tc.tile_pool(name=pname("h3t"), bufs=1) as h3t_pool:
                        h3t = h3t_pool.tile([128, H3 // 128, R], BF,
                                            tag="h3t")
                        with (
                            tc.tile_pool(name=pname("fin"), bufs=1) as fpool,
                            tc.tile_pool(name=pname("finps"), bufs=1,
                                         space="PSUM") as fps,
                        ):
                            aggregate(xw3_fulls, H3, h3t, b3_sb,
                                      post_blk=lambda r: fin_block(
                                          h3t, wlt_sb, r, fpool, fps))

    nc.compile()
    return nc


# ----------------------------------------------------------------------------
# Entry point
# ----------------------------------------------------------------------------

def _make_in_maps(inputs, perm, pre):
    import ml_dtypes
    bf = ml_dtypes.bfloat16
    _, _, _, ab_mats, idx2_tabs, s2_tabs = pre

    def tile_w(w):  # [K, F] -> [128, K/128, F]
        k, f = w.shape
        return np.ascontiguousarray(
            w.reshape(k // 128, 128, f).transpose(1, 0, 2)).astype(bf)

    x_perm = np.asarray(inputs["x"], np.float32)[perm]
    x_tr = np.ascontiguousarray(
        x_perm.reshape(N // 128, 128, F_IN).transpose(1, 0, 2)).astype(bf)
    w1t = tile_w(np.ascontiguousarray(np.asarray(inputs["W1"], np.float32).T))
    w2t = tile_w(np.ascontiguousarray(np.asarray(inputs["W2"], np.float32).T))
    w3t = tile_w(np.ascontiguousarray(np.asarray(inputs["W3"], np.float32).T))
    wlt = tile_w(np.ascontiguousarray(np.asarray(inputs["Wl"], np.float32).T))
    b1pp = np.ascontiguousarray(
        np.asarray(inputs["b1"], np.float32).reshape(-1, 128).T)
    b2 = np.asarray(inputs["b2"], np.float32)
    b3 = np.asarray(inputs["b3"], np.float32)
    zero_b23 = not (b2.any() or b3.any())
    blb = np.ascontiguousarray(
        np.broadcast_to(np.asarray(inputs["bl"], np.float32), (128, F_OUT)))
    ident = np.eye(128, dtype=bf)

    in_maps = []
    for c in range(NUM_CORES):
        m = {
            "x_tr": x_tr, "ab": ab_mats[c].astype(bf),
            "idx2_in": idx2_tabs[c], "s2_in": s2_tabs[c].astype(bf),
            "ident": ident,
            "w1t": w1t, "w2t": w2t, "w3t": w3t, "wlt": wlt,
            "b1pp": b1pp, "blb": blb,
        }
        if not zero_b23:
            m["b2row"] = np.ascontiguousarray(np.broadcast_to(b2, (128, H2)))
            m["b3row"] = np.ascontiguousarray(np.broadcast_to(b3, (128, H3)))
        in_maps.append(m)
    return in_maps, zero_b23


def _run(inputs, trace=False):
    pre = _preprocess(np.asarray(inputs["edge_index"]))
    perm, CH1, CHH = pre[0], pre[1], pre[2]
    in_maps, zero_b23 = _make_in_maps(inputs, perm, pre)
    nc = _build_program(CH1, CHH, zero_b23=zero_b23)
    res = run_bass_kernel_spmd(nc, in_maps, list(range(NUM_CORES)), trace=trace)
    out_perm = np.concatenate([res.results[c]["out"] for c in range(NUM_CORES)], 0)
    out = np.empty_like(out_perm)
    out[perm] = out_perm
    return out, res


def kernel(**inputs):
    out, _ = _run(inputs, trace=False)
    return out
